# revision 1
# baseline (speedup 1.0000x reference)
"""BiLSTM tagger on 8 TRN2 NeuronCores.

Strategy (hardcoded for B=64,T=512,V=30000,E=128,H=256,TAGS=50):
  - Data-parallel: batch sharded 8 ways (8 sequences/core); weights replicated.
  - Per core: embedding gather (indirect DMA) -> PE transpose -> x^T in SBUF;
    input projections xg = W_ih_aug @ [x; 1-m; 1] precomputed for all t as big
    matmuls into DRAM scratch; recurrences (l1 fwd+bwd step-interleaved, then
    l2 fwd+bwd) as dynamic Tile loops, 16 steps per iteration; classifier.
  - Masking: the (1-m) feature adds +/-60 to the f/i gate pre-activations at
    masked steps, freezing c exactly (sigmoid saturates to 1.0/0.0 in fp32).
    Backward-direction h is then exactly 0 at masked steps. Forward l2 output
    h is repaired with copy_predicated (hold previous output where mask=0).
    l1f's garbage masked outputs only feed masked l2 steps, where c is frozen.
  - Gate layout: gates on partitions, chunk order [i0 i1 f0 f1 o0 o1 g0 g1],
    batch on free dim. Per-step xg add done ON THE PE via a bf16 identity
    matmul accumulating into PSUM (h-independent, issues early). PSUM split
    per step into g (computed first, so tanh_g starts while the i/f/o MMs
    still run) and i/f/o banks; sigmoid emitted first on step 0 of each
    body so one combined act-table load serves the whole body. f*c on the
    otherwise-idle GpSimd, rest of the pointwise on DVE, 64-step bodies
    with staggered-reset dynamic loops.
"""
import sys

sys.path.insert(0, "/opt/trn_rl_repo")
import contextlib

import numpy as np
import ml_dtypes

import concourse.bass as bass
import concourse.bacc as bacc
import concourse.mybir as mybir
import concourse.tile as tile
from concourse.bass import ds
from concourse.bass_utils import run_bass_kernel_spmd
from concourse.masks import make_identity

B, T, V, E, H, TAGS = 64, 512, 30000, 128, 256, 50
NCORES = 8
Bc = B // NCORES          # 8 sequences per core
TB = T * Bc               # 4096 tokens per core
STEPS_PER_BODY = 64
NBODY = T // STEPS_PER_BODY  # 8

f32 = mybir.dt.float32
bf16 = mybir.dt.bfloat16
i32 = mybir.dt.int32

UNITS = ("1f", "1b", "2f", "2b")
KCNT = {"1f": 1, "1b": 1, "2f": 4, "2b": 4}       # 128-row K chunks of x features
MCNT = {"1f": 8, "1b": 8, "2f": 10, "2b": 8}      # 128-row output chunks
REV = {"1f": False, "1b": True, "2f": False, "2b": True}

_CACHE = {}


PERM = np.concatenate([np.arange(0, 512), np.arange(768, 1024),
                       np.arange(512, 768)])  # chunk order [i0 i1 f0 f1 o0 o1 g0 g1]


def _prep_unit_weights(Wih, Whh, bih, bhh, m_cnt):
    """Host-side weight marshalling for one LSTM direction.

    Torch row order [i f g o] permuted to chunk order [i i f f o o g g] so
    sigmoid covers contiguous chunks 0:6 and tanh chunks 6:8."""
    din = Wih.shape[1]
    Wp = np.asarray(Wih, np.float32)[PERM]           # [1024, din]
    Up = np.asarray(Whh, np.float32)[PERM]           # [1024, 256]
    bp = (np.asarray(bih) + np.asarray(bhh)).astype(np.float32)[PERM]
    M = m_cnt * 128
    k_cnt = din // 128
    # x-part lhsT: [din, M] -> k-chunk-major cols [128, k_cnt*M]
    WT = np.zeros((din, M), np.float32)
    WT[:, :1024] = Wp.T
    wx = np.concatenate([WT[k * 128:(k + 1) * 128, :] for k in range(k_cnt)],
                        axis=1).astype(ml_dtypes.bfloat16)  # [128, k_cnt*M]
    # aug lhsT rows: feature0 = (1-m), feature1 = 1
    wa = np.zeros((2, M), np.float32)
    wa[0, 0:256] = -60.0   # i rows: -60*(1-m)
    wa[0, 256:512] = 60.0  # f rows: +60*(1-m)
    wa[1, :1024] = bp
    if m_cnt == 10:        # l2f extra planes: m, m (for copy_predicated mask)
        wa[0, 1024:1280] = -1.0
        wa[1, 1024:1280] = 1.0
    wa = wa.astype(ml_dtypes.bfloat16)
    # Whh lhsT: [256, 1024] -> [128, 2*1024]
    UT = Up.T
    wh = np.concatenate([UT[0:128, :], UT[128:256, :]], axis=1).astype(ml_dtypes.bfloat16)
    return wx, wa, wh


def _build_program(stage="full", repeat=1):
    nc = bacc.Bacc("TRN2", target_bir_lowering=False, debug=False, num_devices=NCORES)
    emb_d = nc.dram_tensor("emb", [V, E], f32, kind="ExternalInput")
    words_d = nc.dram_tensor("words", [TB, 1], i32, kind="ExternalInput")
    aug_d = nc.dram_tensor("aug", [2, TB], bf16, kind="ExternalInput")
    wxd, wad, whd, xgd = {}, {}, {}, {}
    for u in UNITS:
        wxd[u] = nc.dram_tensor(f"w{u}x", [128, KCNT[u] * MCNT[u] * 128], bf16, kind="ExternalInput")
        wad[u] = nc.dram_tensor(f"w{u}a", [2, MCNT[u] * 128], bf16, kind="ExternalInput")
        whd[u] = nc.dram_tensor(f"w{u}h", [128, 2048], bf16, kind="ExternalInput")
        xgd[u] = nc.dram_tensor(f"xg{u}", [128, MCNT[u], T, Bc], bf16)
    clsx_d = nc.dram_tensor("clsx", [128, 4 * TAGS], bf16, kind="ExternalInput")
    clsb_d = nc.dram_tensor("clsb", [TAGS, 1], f32, kind="ExternalInput")
    logits_d = nc.dram_tensor("logits", [TAGS, TB], f32, kind="ExternalOutput")

    ctx = contextlib.ExitStack()
    with tile.TileContext(nc) as tc, ctx:
        pp = ctx.enter_context(tc.tile_pool(name="persist", bufs=1))
        xT = pp.tile([128, TB], bf16, tag="xT")
        aug_sb = pp.tile([2, TB], bf16, tag="aug")
        ident = pp.tile([128, 128], f32, tag="ident")
        identb = pp.tile([128, 128], bf16, tag="identb")
        wx_sb = {u: pp.tile([128, KCNT[u] * MCNT[u] * 128], bf16, tag=f"wx{u}", name=f"wx{u}") for u in UNITS}
        wa_sb = {u: pp.tile([2, MCNT[u] * 128], bf16, tag=f"wa{u}", name=f"wa{u}") for u in UNITS}
        wh_sb = {u: pp.tile([128, 2048], bf16, tag=f"wh{u}", name=f"wh{u}") for u in UNITS}
        cls_sb = pp.tile([128, 4 * TAGS], bf16, tag="clsx")
        clsb_sb = pp.tile([TAGS, 1], f32, tag="clsb")
        hs = {u: pp.tile([128, T, 2, Bc], bf16, tag=f"hs{u}", name=f"hs{u}")
              for u in ("1f", "1b", "2b")}
        o2f_sb = pp.tile([128, T, 2, Bc], bf16, tag="o2f")
        hcar = {u: pp.tile([128, 2, Bc], bf16, tag=f"hc{u}", name=f"hc{u}") for u in UNITS}
        ccar = {u: pp.tile([128, 2, Bc], f32, tag=f"cc{u}", name=f"cc{u}") for u in UNITS}
        o2f_run = pp.tile([128, 2, Bc], bf16, tag="o2fr")

        # ---- load weights / constants
        make_identity(nc, ident[:])
        make_identity(nc, identb[:])
        for u in UNITS:
            nc.sync.dma_start(wx_sb[u][:], wxd[u][:])
            nc.sync.dma_start(wa_sb[u][:], wad[u][:])
            nc.sync.dma_start(wh_sb[u][:], whd[u][:])
        nc.sync.dma_start(cls_sb[:], clsx_d[:])
        nc.sync.dma_start(clsb_sb[:], clsb_d[:])
        nc.sync.dma_start(aug_sb[:], aug_d[:])
        for u in UNITS:
            nc.vector.memset(hcar[u][:, :, :], 0.0)
            nc.vector.memset(ccar[u][:, :, :], 0.0)
        nc.vector.memset(o2f_run[:, :, :], 0.0)

        # ---- embedding gather + transpose into xT
        for _rep in range(repeat):
         with nc.named_scope("gather"), \
             tc.tile_pool(name=f"gat{_rep}", bufs=3) as gp, \
             tc.tile_pool(name=f"gps{_rep}", bufs=3, space="PSUM") as gps:
            for n in range(TB // 128):
                idx = gp.tile([128, 1], i32, tag="idx")
                nc.sync.dma_start(idx[:], words_d[n * 128:(n + 1) * 128, :])
                xt = gp.tile([128, 128], f32, tag="xt")
                nc.gpsimd.indirect_dma_start(
                    out=xt[:], out_offset=None, in_=emb_d[:, :],
                    in_offset=bass.IndirectOffsetOnAxis(ap=idx[:, :1], axis=0))
                pst = gps.tile([128, 128], f32, tag="pst")
                nc.tensor.transpose(out=pst[:], in_=xt[:], identity=ident[:])
                nc.vector.tensor_copy(xT[:, n * 128:(n + 1) * 128], pst[:])

        # ---- xg precompute into DRAM, layout [128, m, T, Bc]
        def xg_precompute(u, rhs_of_k, _rep=0):
            m_cnt, k_cnt = MCNT[u], KCNT[u]
            with nc.named_scope(f"xg{u}"), \
                 tc.tile_pool(name=f"xp{u}{_rep}", bufs=4, space="PSUM") as xps, \
                 tc.tile_pool(name=f"xs{u}{_rep}", bufs=4) as xsb:
                for n in range(TB // 512):
                    nsl = slice(n * 512, (n + 1) * 512)
                    for m in range(m_cnt):
                        psm = xps.tile([128, 512], f32, tag="ps")
                        first = True
                        if m < 8:  # gate chunks get the x contribution
                            for k in range(k_cnt):
                                nc.tensor.matmul(
                                    out=psm[:],
                                    lhsT=wx_sb[u][:, (k * m_cnt + m) * 128:(k * m_cnt + m + 1) * 128],
                                    rhs=rhs_of_k(k, n),
                                    start=first, stop=False)
                                first = False
                        nc.tensor.matmul(
                            out=psm[:],
                            lhsT=wa_sb[u][:, m * 128:(m + 1) * 128],
                            rhs=aug_sb[:, nsl],
                            start=first, stop=True)
                        stg = xsb.tile([128, 512], bf16, tag="stg")
                        if (n + m) % 2 == 0:
                            nc.vector.tensor_copy(stg[:], psm[:])
                        else:
                            nc.scalar.activation(stg[:], psm[:],
                                                 mybir.ActivationFunctionType.Copy)
                        nc.sync.dma_start(
                            xgd[u][:, m, n * 64:(n + 1) * 64, :],
                            stg[:, :].rearrange("p (t b) -> p t b", b=Bc))

        def l1_rhs(k, n):
            return xT[:, n * 512:(n + 1) * 512]

        if stage != "gather":
            for _rep in range(repeat):
                xg_precompute("1f", l1_rhs, _rep)
                xg_precompute("1b", l1_rhs, _rep)

        # ---- recurrence phase: the two direction units step-interleaved
        def phase(units, _rep=0):
            with nc.named_scope(f"ph{units[0]}"), \
                 tc.tile_pool(name=f"rc{units[0]}{_rep}", bufs=2) as rp, \
                 tc.tile_pool(name=f"rps{units[0]}{_rep}", bufs=2, space="PSUM") as rps, \
                 tc.tile_pool(name=f"rtmp{units[0]}{_rep}", bufs=4) as tp:
                with tc.For_i(0, NBODY, staggered_reset=True,
                              hint_engines=(mybir.EngineType.PE,
                                            mybir.EngineType.Activation,
                                            mybir.EngineType.DVE)) as i:
                    xb, hstage, t0s = {}, {}, {}
                    ostage = None
                    for u in units:
                        m_cnt = MCNT[u]
                        if REV[u]:
                            t0 = i * (-STEPS_PER_BODY) + (T - STEPS_PER_BODY)
                        else:
                            t0 = i * STEPS_PER_BODY
                        t0s[u] = t0
                        xb[u] = rp.tile([128, m_cnt, STEPS_PER_BODY, Bc], bf16,
                                        tag=f"xb{u}", name=f"xb{u}")
                        nc.sync.dma_start(xb[u][:, :, :, :],
                                          xgd[u][:, :, ds(t0, STEPS_PER_BODY), :])
                        hstage[u] = rp.tile([128, STEPS_PER_BODY, 2, Bc], bf16,
                                            tag=f"hst{u}", name=f"hst{u}")
                        if u == "2f":
                            ostage = rp.tile([128, STEPS_PER_BODY, 2, Bc], bf16,
                                             tag="ost")
                    for us in range(STEPS_PER_BODY):
                        for u in units:
                            rev = REV[u]
                            slot = (STEPS_PER_BODY - 1 - us) if rev else us
                            if us == 0:
                                hprev = hcar[u]
                            else:
                                pslot = slot + 1 if rev else slot - 1
                                hprev = hstage[u][:, pslot, :, :]
                            # separate PSUM banks: g chunks (computed FIRST, so
                            # tanh_g starts while i/f/o MMs still run) and the
                            # sigmoid chunks [i,f,o]
                            psm = rps.tile([128, 6, Bc], f32, tag=f"g{u}")
                            psg = rps.tile([128, 2, Bc], f32, tag=f"gg{u}")
                            # xg add on PE (h-independent, issues early)
                            nc.tensor.matmul(out=psg[:, :, :], lhsT=identb[:],
                                             rhs=xb[u][:, 6:8, slot, :],
                                             start=True, stop=False)
                            nc.tensor.matmul(out=psm[:, :, :], lhsT=identb[:],
                                             rhs=xb[u][:, 0:6, slot, :],
                                             start=True, stop=False)
                            for m in range(6, 8):
                                for k in range(2):
                                    nc.tensor.matmul(
                                        out=psg[:, m - 6, :],
                                        lhsT=wh_sb[u][:, (k * 8 + m) * 128:(k * 8 + m + 1) * 128],
                                        rhs=hprev[:, k, :],
                                        start=False, stop=(m == 7 and k == 1))
                            for m in range(6):
                                for k in range(2):
                                    nc.tensor.matmul(
                                        out=psm[:, m, :],
                                        lhsT=wh_sb[u][:, (k * 8 + m) * 128:(k * 8 + m + 1) * 128],
                                        rhs=hprev[:, k, :],
                                        start=False, stop=(m == 5 and k == 1))
                            tg = tp.tile([128, 2, Bc], f32, tag=f"tg{u}")
                            sg = tp.tile([128, 6, Bc], f32, tag=f"sg{u}")
                            if us == 0:
                                nc.scalar.activation(sg[:, :, :], psm[:, :, :],
                                                     mybir.ActivationFunctionType.Sigmoid)
                                nc.scalar.activation(tg[:, :, :], psg[:, :, :],
                                                     mybir.ActivationFunctionType.Tanh)
                            else:
                                nc.scalar.activation(tg[:, :, :], psg[:, :, :],
                                                     mybir.ActivationFunctionType.Tanh)
                                nc.scalar.activation(sg[:, :, :], psm[:, :, :],
                                                     mybir.ActivationFunctionType.Sigmoid)
                            csf = tp.tile([128, 2, Bc], f32, tag=f"csf{u}")
                            nc.vector.tensor_tensor(out=csf[:, :, :], in0=sg[:, 2:4, :],
                                                    in1=ccar[u][:, :, :], op=mybir.AluOpType.mult)
                            t1 = tp.tile([128, 2, Bc], f32, tag=f"t1{u}")
                            nc.vector.tensor_tensor(out=t1[:, :, :], in0=sg[:, 0:2, :],
                                                    in1=tg[:, :, :], op=mybir.AluOpType.mult)
                            nc.vector.tensor_tensor(out=ccar[u][:, :, :], in0=csf[:, :, :],
                                                    in1=t1[:, :, :], op=mybir.AluOpType.add)
                            tc2 = tp.tile([128, 2, Bc], f32, tag=f"tc2{u}")
                            nc.scalar.activation(tc2[:, :, :], ccar[u][:, :, :],
                                                 mybir.ActivationFunctionType.Tanh)
                            nc.vector.tensor_tensor(out=hstage[u][:, slot, :, :],
                                                    in0=sg[:, 4:6, :],
                                                    in1=tc2[:, :, :], op=mybir.AluOpType.mult)
                            if u == "2f":
                                # running masked output: keep prev where mask=0
                                nc.vector.copy_predicated(o2f_run[:, :, :],
                                                          xb[u][:, 8:10, slot, :].bitcast(mybir.dt.int16),
                                                          hstage[u][:, slot, :, :])
                                nc.vector.tensor_copy(ostage[:, slot, :, :],
                                                      o2f_run[:, :, :])
                    # flush staged h history + carries (on idle GpSimd)
                    for u in units:
                        rev = REV[u]
                        t0 = t0s[u]
                        if u != "2f":
                            nc.vector.tensor_copy(hs[u][:, ds(t0, STEPS_PER_BODY), :, :],
                                                  hstage[u][:, :, :, :])
                        last_slot = 0 if rev else STEPS_PER_BODY - 1
                        nc.vector.tensor_copy(hcar[u][:, :, :],
                                              hstage[u][:, last_slot, :, :])
                        if u == "2f":
                            nc.vector.tensor_copy(o2f_sb[:, ds(t0, STEPS_PER_BODY), :, :],
                                                  ostage[:, :, :, :])

        if stage in ("ph1", "xg2", "full"):
            for _rep in range(repeat):
                phase(("1f", "1b"), _rep)

        def l2_rhs(k, n):
            src = hs["1f"] if k < 2 else hs["1b"]
            return src[:, n * 64:(n + 1) * 64, k % 2, :]

        if stage in ("xg2", "full"):
            for _rep in range(repeat):
                xg_precompute("2f", l2_rhs, _rep)
                xg_precompute("2b", l2_rhs, _rep)

        if stage == "full":
            for _rep in range(repeat):
                phase(("2f", "2b"), _rep)

        # ---- classifier
        if stage != "full":
            with tc.tile_pool(name="dum", bufs=1) as dp:
                dmy = dp.tile([TAGS, 512], f32, tag="dmy")
                nc.vector.memset(dmy[:], 0.0)
                nc.sync.dma_start(logits_d[:, 0:512], dmy[:])
        else:
         with nc.named_scope("cls"), \
             tc.tile_pool(name="cl", bufs=3) as cp, \
             tc.tile_pool(name="cps", bufs=3, space="PSUM") as cps:
            for n in range(TB // 512):
                psm = cps.tile([TAGS, 512], f32, tag="ps")
                for k in range(4):
                    src = o2f_sb if k < 2 else hs["2b"]
                    nc.tensor.matmul(
                        out=psm[:],
                        lhsT=cls_sb[:, k * TAGS:(k + 1) * TAGS],
                        rhs=src[:, n * 64:(n + 1) * 64, k % 2, :],
                        start=(k == 0), stop=(k == 3))
                lg = cp.tile([TAGS, 512], f32, tag="lg")
                nc.vector.tensor_scalar_add(lg[:], psm[:], clsb_sb[:, :1])
                nc.sync.dma_start(logits_d[:, n * 512:(n + 1) * 512], lg[:])

    nc.compile()
    return nc


def _make_in_maps(inputs):
    words = np.asarray(inputs["words"]).astype(np.int32)
    lengths = np.asarray(inputs["lengths"]).astype(np.int32)
    emb = np.asarray(inputs["emb"], dtype=np.float32)
    mask = (lengths[:, None] > np.arange(T)[None, :]).astype(np.float32)
    wprep = {u: _prep_unit_weights(inputs[f"l{u}_Wih"], inputs[f"l{u}_Whh"],
                                   inputs[f"l{u}_bih"], inputs[f"l{u}_bhh"], MCNT[u])
             for u in UNITS}
    clsW = np.asarray(inputs["cls_W"], dtype=np.float32)
    CT = clsW.T
    clsx = np.concatenate([CT[k * 128:(k + 1) * 128, :] for k in range(4)],
                          axis=1).astype(ml_dtypes.bfloat16)
    clsb = np.asarray(inputs["cls_b"], dtype=np.float32).reshape(TAGS, 1)
    in_maps = []
    for c in range(NCORES):
        bsl = slice(c * Bc, (c + 1) * Bc)
        w_c = words[bsl]
        m_c = mask[bsl]
        words_tm = np.ascontiguousarray(w_c.T).reshape(TB, 1)
        aug = np.stack([(1.0 - m_c.T).reshape(TB), np.ones(TB, np.float32)]
                       ).astype(ml_dtypes.bfloat16)
        im = {"emb": emb, "words": words_tm, "aug": aug,
              "clsx": clsx, "clsb": clsb}
        for u in UNITS:
            wx, wa, wh = wprep[u]
            im[f"w{u}x"] = wx
            im[f"w{u}a"] = wa
            im[f"w{u}h"] = wh
        in_maps.append(im)
    return in_maps


def kernel(**inputs):
    if "nc" not in _CACHE:
        _CACHE["nc"] = _build_program()
    nc = _CACHE["nc"]
    in_maps = _make_in_maps(inputs)
    _CACHE["in_maps"] = in_maps
    res = run_bass_kernel_spmd(nc, in_maps, list(range(NCORES)))
    out = np.empty((B, T, TAGS), np.float32)
    for c in range(NCORES):
        lg = res.results[c]["logits"]          # [50, TB], col = t*Bc + b
        out[c * Bc:(c + 1) * Bc] = lg.reshape(TAGS, T, Bc).transpose(2, 1, 0)
    return out


def bench(inputs):
    """Run once with NTFF tracing; returns HW exec_time_ns (and stashes trace)."""
    kernel(**inputs)  # ensure program built/cached
    nc = _CACHE["nc"]
    in_maps = _CACHE["in_maps"]
    import tempfile
    tmpdir = tempfile.mkdtemp(prefix="bilstm_trace_")
    res = run_bass_kernel_spmd(nc, in_maps, list(range(NCORES)), trace=True,
                               tmpdir=tmpdir)
    _CACHE["trace_dir"] = tmpdir
    _CACHE["last_bench"] = res
    print("trace dir:", tmpdir)
    if res.per_core_scope_times:
        for scope, times in res.per_core_scope_times.items():
            print(f"scope {scope}: {times}")
    return res.exec_time_ns


if __name__ == "__main__":
    import reference
    inputs = {k: np.asarray(v) for k, v in reference.setup_inputs().items()}
    got = kernel(**inputs)
    print(got.shape, got.dtype)



# revision 3
# speedup vs baseline: 1.0911x; 1.0911x over previous
"""BiLSTM tagger on 8 TRN2 NeuronCores.

Strategy (hardcoded for B=64,T=512,V=30000,E=128,H=256,TAGS=50):
  - Data-parallel: batch sharded 8 ways (8 sequences/core); weights replicated.
  - Per core: embedding gather (indirect DMA) -> PE transpose -> x^T in SBUF;
    input projections xg = W_ih_aug @ [x; 1-m; 1] precomputed for all t as big
    matmuls into DRAM scratch; recurrences (l1 fwd+bwd step-interleaved, then
    l2 fwd+bwd) as dynamic Tile loops, 64 steps per body; classifier.
  - ALL-SIGMOID formulation: tanh(x) = 2*sigmoid(2x)-1 folded into weight
    scaling. The carried state is h/2 and c/2; consumers' weights are
    pre-scaled by 2 (Whh, l2 Wih, cls_W), g-gate rows by an extra 2. Every
    activation instruction is Sigmoid (tanh(c) = sigma(4*(c/2)) via the
    activation scale arg) -> zero act-table switches and only 2 activation
    instructions per unit-step: sigma(all 8 gate chunks), sigma(4c').
  - Per unit-step: DVE prefills PSUM with xg (h-independent, off critical
    path); 16 Whh matmuls accumulate (start=False); Act sigma_all; GpSimd
    csf = sf*c'; DVE u1 = (sg-.5)*si; DVE c' = csf+u1; Act sigma(4c');
    DVE h' = (sc-.5)*so -> staged bf16.
  - Masking: +/-60*(1-m) on i/f gate pre-activations freezes c exactly at
    masked steps; backward h is exactly 0 there. The l2f held output is
    reconstructed AFTER the recurrence with tensor_tensor_scan along t:
    held[t] = (1-m[t])*held[t-1] + m[t]*h[t] (16 scan instrs instead of
    1024 in-loop predicated copies).
"""
import sys

sys.path.insert(0, "/opt/trn_rl_repo")
import contextlib

import numpy as np
import ml_dtypes

import concourse.bass as bass
import concourse.bacc as bacc
import concourse.mybir as mybir
import concourse.tile as tile
from concourse.bass import ds
from concourse.bass_utils import run_bass_kernel_spmd
from concourse.masks import make_identity

B, T, V, E, H, TAGS = 64, 512, 30000, 128, 256, 50
NCORES = 8
Bc = B // NCORES          # 8 sequences per core
TB = T * Bc               # 4096 tokens per core
STEPS_PER_BODY = 64
NBODY = T // STEPS_PER_BODY  # 8

f32 = mybir.dt.float32
bf16 = mybir.dt.bfloat16
i32 = mybir.dt.int32

UNITS = ("1f", "1b", "2f", "2b")
KCNT = {"1f": 1, "1b": 1, "2f": 4, "2b": 4}       # 128-row K chunks of x features
MCNT = {"1f": 8, "1b": 8, "2f": 10, "2b": 8}      # 128-row output chunks
REV = {"1f": False, "1b": True, "2f": False, "2b": True}

_CACHE = {}

SIG = mybir.ActivationFunctionType.Sigmoid

# permuted gate-row order: [i(0:256), f(256:512), o(512:768), g(768:1024)]
PERM = np.concatenate([np.arange(0, 512), np.arange(768, 1024),
                       np.arange(512, 768)])


def _prep_unit_weights(Wih, Whh, bih, bhh, m_cnt, in_scale):
    """Host-side weight marshalling for one LSTM direction (all-sigmoid form).

    in_scale compensates h/2-scaled inputs (2.0 for l2 units). Whh is scaled
    by 2 (recurrent h is h/2); g rows by an extra 2 (tanh = 2*sig(2x)-1)."""
    din = Wih.shape[1]
    Wp = np.asarray(Wih, np.float64)[PERM] * in_scale   # [1024, din]
    Up = np.asarray(Whh, np.float64)[PERM] * 2.0        # [1024, 256]
    bp = (np.asarray(bih, np.float64) + np.asarray(bhh, np.float64))[PERM]
    Wp[768:1024] *= 2.0
    Up[768:1024] *= 2.0
    bp = bp.copy()
    bp[768:1024] *= 2.0
    M = m_cnt * 128
    k_cnt = din // 128
    # x-part lhsT: [din, M] -> k-chunk-major cols [128, k_cnt*M]
    WT = np.zeros((din, M), np.float64)
    WT[:, :1024] = Wp.T
    wx = np.concatenate([WT[k * 128:(k + 1) * 128, :] for k in range(k_cnt)],
                        axis=1).astype(ml_dtypes.bfloat16)  # [128, k_cnt*M]
    # aug lhsT rows: feature0 = (1-m), feature1 = 1
    wa = np.zeros((2, M), np.float64)
    wa[0, 0:256] = -60.0   # i rows: -60*(1-m)
    wa[0, 256:512] = 60.0  # f rows: +60*(1-m)
    wa[1, :1024] = bp
    if m_cnt == 10:        # l2f extra planes: chunk8 = m, chunk9 = 1-m
        wa[0, 1024:1152] = -1.0
        wa[1, 1024:1152] = 1.0
        wa[0, 1152:1280] = 1.0
        wa[1, 1152:1280] = 0.0
    wa = wa.astype(ml_dtypes.bfloat16)
    # Whh lhsT: [256, 1024] -> [128, 2*1024], (k*8+m) chunk indexing
    UT = Up.T
    wh = np.concatenate([UT[0:128, :], UT[128:256, :]], axis=1).astype(ml_dtypes.bfloat16)
    return wx, wa, wh


def _build_program():
    nc = bacc.Bacc("TRN2", target_bir_lowering=False, debug=False, num_devices=NCORES)
    emb_d = nc.dram_tensor("emb", [V, E], f32, kind="ExternalInput")
    words_d = nc.dram_tensor("words", [TB, 1], i32, kind="ExternalInput")
    aug_d = nc.dram_tensor("aug", [2, TB], bf16, kind="ExternalInput")
    wxd, wad, whd, xgd = {}, {}, {}, {}
    for u in UNITS:
        wxd[u] = nc.dram_tensor(f"w{u}x", [128, KCNT[u] * MCNT[u] * 128], bf16, kind="ExternalInput")
        wad[u] = nc.dram_tensor(f"w{u}a", [2, MCNT[u] * 128], bf16, kind="ExternalInput")
        whd[u] = nc.dram_tensor(f"w{u}h", [128, 2048], bf16, kind="ExternalInput")
        xgd[u] = nc.dram_tensor(f"xg{u}", [128, MCNT[u], T, Bc], bf16)
    clsx_d = nc.dram_tensor("clsx", [128, 4 * TAGS], bf16, kind="ExternalInput")
    clsb_d = nc.dram_tensor("clsb", [TAGS, 1], f32, kind="ExternalInput")
    logits_d = nc.dram_tensor("logits", [TAGS, TB], f32, kind="ExternalOutput")

    ctx = contextlib.ExitStack()
    with tile.TileContext(nc) as tc, ctx:
        pp = ctx.enter_context(tc.tile_pool(name="persist", bufs=1))
        aug_sb = pp.tile([2, TB], bf16, tag="aug")
        wh_sb = {u: pp.tile([128, 2048], bf16, tag=f"wh{u}", name=f"wh{u}") for u in UNITS}
        cls_sb = pp.tile([128, 4 * TAGS], bf16, tag="clsx")
        clsb_sb = pp.tile([TAGS, 1], f32, tag="clsb")
        # h histories (bf16, h/2 scale). 2f is RAW (pre-hold); o2f is the
        # scan-reconstructed held output.
        hs = {u: pp.tile([128, T, 2, Bc], bf16, tag=f"hs{u}", name=f"hs{u}")
              for u in ("1f", "1b", "2f", "2b")}
        o2f_sb = pp.tile([128, T, 2, Bc], bf16, tag="o2f")
        hcar = {u: pp.tile([128, 2, Bc], bf16, tag=f"hc{u}", name=f"hc{u}") for u in UNITS}
        ccar = {u: pp.tile([128, 2, Bc], f32, tag=f"cc{u}", name=f"cc{u}") for u in UNITS}

        # ---- load persistent weights / init carries
        for u in UNITS:
            nc.sync.dma_start(wh_sb[u][:], whd[u][:])
        nc.sync.dma_start(cls_sb[:], clsx_d[:])
        nc.sync.dma_start(clsb_sb[:], clsb_d[:])
        nc.sync.dma_start(aug_sb[:], aug_d[:])
        for u in UNITS:
            nc.vector.memset(hcar[u][:, :, :], 0.0)
            nc.vector.memset(ccar[u][:, :, :], 0.0)

        # ---- prologue scope: gather + xg1 (xT and wx1 freed afterwards)
        with nc.named_scope("gather"), \
             tc.tile_pool(name="prolog", bufs=1) as lp, \
             tc.tile_pool(name="gat", bufs=3) as gp, \
             tc.tile_pool(name="gps", bufs=3, space="PSUM") as gps:
            xT = lp.tile([128, TB], bf16, tag="xT")
            ident = lp.tile([128, 128], f32, tag="ident")
            make_identity(nc, ident[:])
            wx_sb1 = {}
            for u in ("1f", "1b"):
                wx_sb1[u] = lp.tile([128, KCNT[u] * MCNT[u] * 128], bf16, tag=f"wx{u}", name=f"wx{u}")
                nc.sync.dma_start(wx_sb1[u][:], wxd[u][:])
                wa_sb = lp.tile([2, MCNT[u] * 128], bf16, tag=f"wa{u}", name=f"wa{u}")
                nc.sync.dma_start(wa_sb[:], wad[u][:])
                wx_sb1[u + "a"] = wa_sb
            for n in range(TB // 128):
                idx = gp.tile([128, 1], i32, tag="idx")
                nc.sync.dma_start(idx[:], words_d[n * 128:(n + 1) * 128, :])
                xt = gp.tile([128, 128], f32, tag="xt")
                nc.gpsimd.indirect_dma_start(
                    out=xt[:], out_offset=None, in_=emb_d[:, :],
                    in_offset=bass.IndirectOffsetOnAxis(ap=idx[:, :1], axis=0))
                pst = gps.tile([128, 128], f32, tag="pst")
                nc.tensor.transpose(out=pst[:], in_=xt[:], identity=ident[:])
                nc.vector.tensor_copy(xT[:, n * 128:(n + 1) * 128], pst[:])

            # xg1 for both l1 directions
            def l1_rhs(k, n):
                return xT[:, n * 512:(n + 1) * 512]
            for u in ("1f", "1b"):
                xg_precompute_body(nc, tc, u, l1_rhs, wx_sb1[u], wx_sb1[u + "a"],
                                   aug_sb, xgd[u])

        # ---- recurrence phase: the two direction units step-interleaved
        def phase(units):
            with nc.named_scope(f"ph{units[0]}"), \
                 tc.tile_pool(name=f"rc{units[0]}", bufs=2) as rp, \
                 tc.tile_pool(name=f"rps{units[0]}", bufs=2, space="PSUM") as rps, \
                 tc.tile_pool(name=f"rtmp{units[0]}", bufs=4) as tp:
                with tc.For_i(0, NBODY, staggered_reset=True,
                              hint_engines=(mybir.EngineType.PE,
                                            mybir.EngineType.Activation,
                                            mybir.EngineType.DVE,
                                            mybir.EngineType.Pool)) as i:
                    xb, hstage, t0s, psum = {}, {}, {}, {}
                    for u in units:
                        if REV[u]:
                            t0 = i * (-STEPS_PER_BODY) + (T - STEPS_PER_BODY)
                        else:
                            t0 = i * STEPS_PER_BODY
                        t0s[u] = t0
                        xb[u] = rp.tile([128, 8, STEPS_PER_BODY, Bc], bf16,
                                        tag=f"xb{u}", name=f"xb{u}")
                        nc.sync.dma_start(xb[u][:, :, :, :],
                                          xgd[u][:, 0:8, ds(t0, STEPS_PER_BODY), :])
                        hstage[u] = rp.tile([128, STEPS_PER_BODY, 2, Bc], bf16,
                                            tag=f"hst{u}", name=f"hst{u}")
                        psum[u] = [None, None]
                    # prefill PSUM with xg for the first step of each unit
                    for u in units:
                        slot0 = (STEPS_PER_BODY - 1) if REV[u] else 0
                        ps = rps.tile([128, 8, Bc], f32, tag=f"ps{u}", name=f"ps{u}")
                        nc.vector.tensor_copy(ps[:, :, :], xb[u][:, :, slot0, :])
                        psum[u][0] = ps
                    for us in range(STEPS_PER_BODY):
                        for u in units:
                            rev = REV[u]
                            slot = (STEPS_PER_BODY - 1 - us) if rev else us
                            if us == 0:
                                hprev = hcar[u]
                            else:
                                pslot = slot + 1 if rev else slot - 1
                                hprev = hstage[u][:, pslot, :, :]
                            psm = psum[u][us % 2]
                            # 16 Whh matmuls accumulate onto the prefilled xg
                            for m in range(8):
                                for k in range(2):
                                    nc.tensor.matmul(
                                        out=psm[:, m, :],
                                        lhsT=wh_sb[u][:, (k * 8 + m) * 128:(k * 8 + m + 1) * 128],
                                        rhs=hprev[:, k, :],
                                        start=False, stop=(k == 1),
                                        skip_group_check=True)
                            # prefill the NEXT step's PSUM (h-independent)
                            if us + 1 < STEPS_PER_BODY:
                                nslot = (slot - 1) if rev else (slot + 1)
                                ps = rps.tile([128, 8, Bc], f32, tag=f"ps{u}", name=f"ps{u}")
                                nc.vector.tensor_copy(ps[:, :, :],
                                                      xb[u][:, :, nslot, :])
                                psum[u][(us + 1) % 2] = ps
                            # pointwise tail (all-sigmoid)
                            sg = tp.tile([128, 8, Bc], f32, tag=f"sg{u}")
                            nc.scalar.activation(sg[:, :, :], psm[:, :, :], SIG)
                            csf = tp.tile([128, 2, Bc], f32, tag=f"csf{u}")
                            nc.gpsimd.tensor_tensor(out=csf[:, :, :], in0=sg[:, 2:4, :],
                                                    in1=ccar[u][:, :, :],
                                                    op=mybir.AluOpType.mult)
                            u1 = tp.tile([128, 2, Bc], f32, tag=f"u1{u}")
                            nc.vector.scalar_tensor_tensor(
                                out=u1[:, :, :], in0=sg[:, 6:8, :], scalar=-0.5,
                                in1=sg[:, 0:2, :],
                                op0=mybir.AluOpType.add, op1=mybir.AluOpType.mult)
                            nc.vector.tensor_tensor(out=ccar[u][:, :, :],
                                                    in0=csf[:, :, :], in1=u1[:, :, :],
                                                    op=mybir.AluOpType.add)
                            sc = tp.tile([128, 2, Bc], f32, tag=f"sc{u}")
                            nc.scalar.activation(sc[:, :, :], ccar[u][:, :, :],
                                                 SIG, scale=4.0)
                            nc.vector.scalar_tensor_tensor(
                                out=hstage[u][:, slot, :, :], in0=sc[:, :, :],
                                scalar=-0.5, in1=sg[:, 4:6, :],
                                op0=mybir.AluOpType.add, op1=mybir.AluOpType.mult)
                    # flush staged h history + carries (on idle GpSimd)
                    for u in units:
                        rev = REV[u]
                        t0 = t0s[u]
                        nc.gpsimd.tensor_copy(hs[u][:, ds(t0, STEPS_PER_BODY), :, :],
                                              hstage[u][:, :, :, :])
                        last_slot = 0 if rev else STEPS_PER_BODY - 1
                        nc.gpsimd.tensor_copy(hcar[u][:, :, :],
                                              hstage[u][:, last_slot, :, :])

        phase(("1f", "1b"))

        # ---- xg2 (consumes hs1f/hs1b)
        def l2_rhs(k, n):
            src = hs["1f"] if k < 2 else hs["1b"]
            return src[:, n * 64:(n + 1) * 64, k % 2, :]

        with nc.named_scope("xg2"), tc.tile_pool(name="xg2w", bufs=1) as xwp:
            for u in ("2f", "2b"):
                wx_sb = xwp.tile([128, KCNT[u] * MCNT[u] * 128], bf16, tag=f"wx{u}", name=f"wx2{u}")
                nc.sync.dma_start(wx_sb[:], wxd[u][:])
                wa_sb = xwp.tile([2, MCNT[u] * 128], bf16, tag=f"wa{u}", name=f"wa2{u}")
                nc.sync.dma_start(wa_sb[:], wad[u][:])
                xg_precompute_body(nc, tc, u, l2_rhs, wx_sb, wa_sb, aug_sb, xgd[u])

        phase(("2f", "2b"))

        # ---- scan-hold: o2f[t] = (1-m[t])*o2f[t-1] + m[t]*h2f[t]
        with nc.named_scope("hold"), tc.tile_pool(name="hold", bufs=1) as hp:
            m_sb = hp.tile([128, 2, T, Bc], bf16, tag="msb")   # [m, 1-m] planes
            nc.sync.dma_start(m_sb[:, :, :, :], xgd["2f"][:, 8:10, :, :])
            tmp = hp.tile([128, T, Bc], bf16, tag="tmp")
            for k in range(2):
                nc.vector.tensor_tensor(out=tmp[:, :, :], in0=hs["2f"][:, :, k, :],
                                        in1=m_sb[:, 0, :, :], op=mybir.AluOpType.mult)
                for b in range(Bc):
                    nc.vector.tensor_tensor_scan(
                        out=o2f_sb[:, :, k, b], data0=m_sb[:, 1, :, b],
                        data1=tmp[:, :, b], initial=0.0,
                        op0=mybir.AluOpType.mult, op1=mybir.AluOpType.add)

        # ---- classifier
        with nc.named_scope("cls"), \
             tc.tile_pool(name="cl", bufs=3) as cp, \
             tc.tile_pool(name="cps", bufs=3, space="PSUM") as cps:
            for n in range(TB // 512):
                psm = cps.tile([TAGS, 512], f32, tag="ps")
                for k in range(4):
                    src = o2f_sb if k < 2 else hs["2b"]
                    nc.tensor.matmul(
                        out=psm[:],
                        lhsT=cls_sb[:, k * TAGS:(k + 1) * TAGS],
                        rhs=src[:, n * 64:(n + 1) * 64, k % 2, :],
                        start=(k == 0), stop=(k == 3))
                lg = cp.tile([TAGS, 512], f32, tag="lg")
                nc.vector.tensor_scalar_add(lg[:], psm[:], clsb_sb[:, :1])
                nc.sync.dma_start(logits_d[:, n * 512:(n + 1) * 512], lg[:])

    nc.compile()
    return nc


def xg_precompute_body(nc, tc, u, rhs_of_k, wx_sb, wa_sb, aug_sb, xg_dram):
    """xg precompute into DRAM, layout [128, m, T, Bc]."""
    m_cnt, k_cnt = MCNT[u], KCNT[u]
    with nc.named_scope(f"xg{u}"), \
         tc.tile_pool(name=f"xp{u}", bufs=4, space="PSUM") as xps, \
         tc.tile_pool(name=f"xs{u}", bufs=4) as xsb:
        for n in range(TB // 512):
            nsl = slice(n * 512, (n + 1) * 512)
            for m in range(m_cnt):
                psm = xps.tile([128, 512], f32, tag="ps")
                first = True
                if m < 8:  # gate chunks get the x contribution
                    for k in range(k_cnt):
                        nc.tensor.matmul(
                            out=psm[:],
                            lhsT=wx_sb[:, (k * m_cnt + m) * 128:(k * m_cnt + m + 1) * 128],
                            rhs=rhs_of_k(k, n),
                            start=first, stop=False)
                        first = False
                nc.tensor.matmul(
                    out=psm[:],
                    lhsT=wa_sb[:, m * 128:(m + 1) * 128],
                    rhs=aug_sb[:, nsl],
                    start=first, stop=True)
                stg = xsb.tile([128, 512], bf16, tag="stg")
                if (n + m) % 2 == 0:
                    nc.vector.tensor_copy(stg[:], psm[:])
                else:
                    nc.scalar.activation(stg[:], psm[:],
                                         mybir.ActivationFunctionType.Copy)
                nc.sync.dma_start(
                    xg_dram[:, m, n * 64:(n + 1) * 64, :],
                    stg[:, :].rearrange("p (t b) -> p t b", b=Bc))


def _make_in_maps(inputs):
    words = np.asarray(inputs["words"]).astype(np.int32)
    lengths = np.asarray(inputs["lengths"]).astype(np.int32)
    emb = np.asarray(inputs["emb"], dtype=np.float32)
    mask = (lengths[:, None] > np.arange(T)[None, :]).astype(np.float32)
    wprep = {u: _prep_unit_weights(inputs[f"l{u}_Wih"], inputs[f"l{u}_Whh"],
                                   inputs[f"l{u}_bih"], inputs[f"l{u}_bhh"],
                                   MCNT[u], 2.0 if u[0] == "2" else 1.0)
             for u in UNITS}
    clsW = np.asarray(inputs["cls_W"], np.float64) * 2.0
    CT = clsW.T
    clsx = np.concatenate([CT[k * 128:(k + 1) * 128, :] for k in range(4)],
                          axis=1).astype(ml_dtypes.bfloat16)
    clsb = np.asarray(inputs["cls_b"], dtype=np.float32).reshape(TAGS, 1)
    in_maps = []
    for c in range(NCORES):
        bsl = slice(c * Bc, (c + 1) * Bc)
        w_c = words[bsl]
        m_c = mask[bsl]
        words_tm = np.ascontiguousarray(w_c.T).reshape(TB, 1)
        aug = np.stack([(1.0 - m_c.T).reshape(TB), np.ones(TB, np.float32)]
                       ).astype(ml_dtypes.bfloat16)
        im = {"emb": emb, "words": words_tm, "aug": aug,
              "clsx": clsx, "clsb": clsb}
        for u in UNITS:
            wx, wa, wh = wprep[u]
            im[f"w{u}x"] = wx
            im[f"w{u}a"] = wa
            im[f"w{u}h"] = wh
        in_maps.append(im)
    return in_maps


def kernel(**inputs):
    if "nc" not in _CACHE:
        _CACHE["nc"] = _build_program()
    nc = _CACHE["nc"]
    in_maps = _make_in_maps(inputs)
    _CACHE["in_maps"] = in_maps
    res = run_bass_kernel_spmd(nc, in_maps, list(range(NCORES)))
    out = np.empty((B, T, TAGS), np.float32)
    for c in range(NCORES):
        lg = res.results[c]["logits"]          # [50, TB], col = t*Bc + b
        out[c * Bc:(c + 1) * Bc] = lg.reshape(TAGS, T, Bc).transpose(2, 1, 0)
    return out


def bench(inputs):
    """Run once with NTFF tracing; returns HW exec_time_ns (and stashes trace)."""
    kernel(**inputs)  # ensure program built/cached
    nc = _CACHE["nc"]
    in_maps = _CACHE["in_maps"]
    import tempfile
    tmpdir = tempfile.mkdtemp(prefix="bilstm_trace_")
    res = run_bass_kernel_spmd(nc, in_maps, list(range(NCORES)), trace=True,
                               tmpdir=tmpdir)
    _CACHE["trace_dir"] = tmpdir
    _CACHE["last_bench"] = res
    print("trace dir:", tmpdir)
    if res.per_core_scope_times:
        for scope, times in res.per_core_scope_times.items():
            print(f"scope {scope}: {times}")
    return res.exec_time_ns


if __name__ == "__main__":
    import reference
    inputs = {k: np.asarray(v) for k, v in reference.setup_inputs().items()}
    got = kernel(**inputs)
    print(got.shape, got.dtype)


# revision 7
# speedup vs baseline: 1.1890x; 1.0898x over previous
"""BiLSTM tagger on 8 TRN2 NeuronCores.

Strategy (hardcoded for B=64,T=512,V=30000,E=128,H=256,TAGS=50):
  - Data-parallel: batch sharded 8 ways (8 sequences/core); weights replicated.
  - Per core: embedding gather (indirect DMA) -> PE transpose -> x^T in SBUF;
    input projections xg = W_ih_aug @ [x; 1-m; 1] precomputed for all t as big
    matmuls into DRAM scratch (middle-out tile order so ph1 can start after
    2 tiles); recurrences fully STATICALLY UNROLLED (no dynamic loops), xb
    DMAs prefetched one 64-step body ahead; classifier.
  - ALL-SIGMOID formulation: tanh(x) = 2*sigmoid(2x)-1 folded into weight
    scaling. Carried state is h/2, c/2; consumers' weights pre-scaled by 2
    (Whh, l2 Wih, cls_W), g-gate rows by an extra 2. Every activation is
    Sigmoid (tanh(c) = sig(4*(c/2)) via the activation scale arg) -> zero
    act-table switches.
  - Per unit-step: DVE prefills PSUM with xg (h-independent, off critical
    path); 12 Whh matmuls for i,f,g chunks then sig(ifg) fires while the 4
    o-chunk matmuls still run; DVE u1=(sg-.5)*si, csf=sf*c', c'=csf+u1;
    Act sig(4c'); DVE h'=(sc-.5)*so -> staged bf16.
  - Masking: +/-60*(1-m) on i/f gate pre-activations freezes c exactly at
    masked steps; backward h is exactly 0 there. The l2f held output is
    reconstructed per-body with tensor_tensor_scan along t:
    held[t] = (1-m[t])*held[t-1] + m[t]*h[t].
"""
import sys

sys.path.insert(0, "/opt/trn_rl_repo")
import contextlib

import numpy as np
import ml_dtypes

import concourse.bass as bass
import concourse.bacc as bacc
import concourse.mybir as mybir
import concourse.tile as tile
from concourse.bass_utils import run_bass_kernel_spmd
from concourse.masks import make_identity

B, T, V, E, H, TAGS = 64, 512, 30000, 128, 256, 50
NCORES = 8
Bc = B // NCORES          # 8 sequences per core
TB = T * Bc               # 4096 tokens per core
SPB = 64                  # steps per body
NBODY = T // SPB          # 8

f32 = mybir.dt.float32
bf16 = mybir.dt.bfloat16
i32 = mybir.dt.int32

UNITS = ("1f", "1b", "2f", "2b")
KCNT = {"1f": 1, "1b": 1, "2f": 4, "2b": 4}       # 128-row K chunks of x features
MCNT = {"1f": 8, "1b": 8, "2f": 10, "2b": 8}      # 128-row output chunks
REV = {"1f": False, "1b": True, "2f": False, "2b": True}

_CACHE = {}

SIG = mybir.ActivationFunctionType.Sigmoid
ADD = mybir.AluOpType.add
MUL = mybir.AluOpType.mult

# gate-row order [i(0:256), f(256:512), g(512:768), o(768:1024)] = torch order
# with g and o swapped
PERM = np.concatenate([np.arange(0, 512), np.arange(512, 768),
                       np.arange(768, 1024)])  # identity on i,f; then g; then o


def _prep_unit_weights(Wih, Whh, bih, bhh, m_cnt, in_scale):
    """Host-side weight marshalling (all-sigmoid form).

    Torch row order is [i f g o]; we keep it (i=chunks0:2, f=2:4, g=4:6,
    o=6:8). in_scale compensates h/2-scaled inputs (2.0 for l2). Whh x2
    (recurrent h is h/2); g rows an extra x2 (tanh = 2*sig(2x)-1)."""
    din = Wih.shape[1]
    Wp = np.asarray(Wih, np.float64) * in_scale   # [1024, din]
    Up = np.asarray(Whh, np.float64) * 2.0        # [1024, 256]
    bp = (np.asarray(bih, np.float64) + np.asarray(bhh, np.float64)).copy()
    Wp = Wp.copy()
    Wp[512:768] *= 2.0
    Up = Up.copy()
    Up[512:768] *= 2.0
    bp[512:768] *= 2.0
    M = m_cnt * 128
    k_cnt = din // 128
    # x-part lhsT: [din, M] -> k-chunk-major cols [128, k_cnt*M]
    WT = np.zeros((din, M), np.float64)
    WT[:, :1024] = Wp.T
    wx = np.concatenate([WT[k * 128:(k + 1) * 128, :] for k in range(k_cnt)],
                        axis=1).astype(ml_dtypes.bfloat16)  # [128, k_cnt*M]
    # aug lhsT rows: feature0 = (1-m), feature1 = 1
    wa = np.zeros((2, M), np.float64)
    wa[0, 0:256] = -60.0   # i rows: -60*(1-m)
    wa[0, 256:512] = 60.0  # f rows: +60*(1-m)
    wa[1, :1024] = bp
    if m_cnt == 10:        # l2f extra planes: chunk8 = m, chunk9 = 1-m
        wa[0, 1024:1152] = -1.0
        wa[1, 1024:1152] = 1.0
        wa[0, 1152:1280] = 1.0
        wa[1, 1152:1280] = 0.0
    wa = wa.astype(ml_dtypes.bfloat16)
    # Whh lhsT: [256, 1024] -> [128, 2*1024], (k*8+m) chunk indexing
    UT = Up.T
    wh = np.concatenate([UT[0:128, :], UT[128:256, :]], axis=1).astype(ml_dtypes.bfloat16)
    return wx, wa, wh


def _build_program():
    nc = bacc.Bacc("TRN2", target_bir_lowering=False, debug=False, num_devices=NCORES)
    emb_d = nc.dram_tensor("emb", [V, E], f32, kind="ExternalInput")
    words_d = nc.dram_tensor("words", [TB, 1], i32, kind="ExternalInput")
    aug_d = nc.dram_tensor("aug", [2, TB], bf16, kind="ExternalInput")
    wxd, wad, whd, xgd = {}, {}, {}, {}
    for u in UNITS:
        wxd[u] = nc.dram_tensor(f"w{u}x", [128, KCNT[u] * MCNT[u] * 128], bf16, kind="ExternalInput")
        wad[u] = nc.dram_tensor(f"w{u}a", [2, MCNT[u] * 128], bf16, kind="ExternalInput")
        whd[u] = nc.dram_tensor(f"w{u}h", [128, 2048], bf16, kind="ExternalInput")
        xgd[u] = nc.dram_tensor(f"xg{u}", [128, MCNT[u], T, Bc], bf16)
    clsx_d = nc.dram_tensor("clsx", [128, 4 * TAGS], bf16, kind="ExternalInput")
    clsb_d = nc.dram_tensor("clsb", [TAGS, 1], f32, kind="ExternalInput")
    logits_d = nc.dram_tensor("logits", [TAGS, TB], f32, kind="ExternalOutput")

    ctx = contextlib.ExitStack()
    with tile.TileContext(nc) as tc, ctx:
        pp = ctx.enter_context(tc.tile_pool(name="persist", bufs=1))
        aug_sb = pp.tile([2, TB], bf16, tag="aug")
        wh_sb = {u: pp.tile([128, 2048], bf16, tag=f"wh{u}", name=f"wh{u}") for u in UNITS}
        cls_sb = pp.tile([128, 4 * TAGS], bf16, tag="clsx")
        clsb_sb = pp.tile([TAGS, 1], f32, tag="clsb")
        hs = {u: pp.tile([128, T, 2, Bc], bf16, tag=f"hs{u}", name=f"hs{u}")
              for u in ("1f", "1b", "2b")}
        o2f = pp.tile([128, T + 1, 2, Bc], bf16, tag="o2f")  # col0 = zeros
        hcar = {u: pp.tile([128, 2, Bc], bf16, tag=f"hc{u}", name=f"hc{u}") for u in UNITS}
        ccar = {u: pp.tile([128, 2, Bc], f32, tag=f"cc{u}", name=f"cc{u}") for u in UNITS}

        for u in UNITS:
            nc.sync.dma_start(wh_sb[u][:], whd[u][:])
        nc.sync.dma_start(cls_sb[:], clsx_d[:])
        nc.sync.dma_start(clsb_sb[:], clsb_d[:])
        nc.sync.dma_start(aug_sb[:], aug_d[:])
        for u in UNITS:
            nc.vector.memset(hcar[u][:, :, :], 0.0)
            nc.vector.memset(ccar[u][:, :, :], 0.0)
        nc.vector.memset(o2f[:, 0, :, :], 0.0)

        # ---- prologue: gather + xg1 interleaved, middle-out tile order
        with nc.named_scope("gather"), \
             tc.tile_pool(name="prolog", bufs=1) as lp, \
             tc.tile_pool(name="gat", bufs=4) as gp, \
             tc.tile_pool(name="gps", bufs=4, space="PSUM") as gps, \
             tc.tile_pool(name="xp1", bufs=4, space="PSUM") as xps, \
             tc.tile_pool(name="xs1", bufs=4) as xsb:
            xT = lp.tile([128, TB], bf16, tag="xT")
            ident = lp.tile([128, 128], f32, tag="ident")
            make_identity(nc, ident[:])
            wx1, wa1 = {}, {}
            for u in ("1f", "1b"):
                wx1[u] = lp.tile([128, KCNT[u] * MCNT[u] * 128], bf16,
                                 tag=f"wx{u}", name=f"wx{u}")
                nc.sync.dma_start(wx1[u][:], wxd[u][:])
                wa1[u] = lp.tile([2, MCNT[u] * 128], bf16, tag=f"wa{u}",
                                 name=f"wa{u}")
                nc.sync.dma_start(wa1[u][:], wad[u][:])

            def l1_rhs(k, n):
                return xT[:, n * 512:(n + 1) * 512]

            order = [0, 7, 1, 6, 2, 5, 3, 4]
            for tI in order:
                for j in range(4):
                    n = tI * 4 + j
                    idx = gp.tile([128, 1], i32, tag="idx")
                    nc.sync.dma_start(idx[:], words_d[n * 128:(n + 1) * 128, :])
                    xt = gp.tile([128, 128], f32, tag="xt")
                    nc.gpsimd.indirect_dma_start(
                        out=xt[:], out_offset=None, in_=emb_d[:, :],
                        in_offset=bass.IndirectOffsetOnAxis(ap=idx[:, :1], axis=0))
                    pst = gps.tile([128, 128], f32, tag="pst")
                    nc.tensor.transpose(out=pst[:], in_=xt[:], identity=ident[:])
                    nc.vector.tensor_copy(xT[:, n * 128:(n + 1) * 128], pst[:])
                for u in ("1f", "1b"):
                    xg_tile(nc, u, tI, l1_rhs, wx1[u], wa1[u], aug_sb, xgd[u],
                            xps, xsb)

        phase(nc, tc, ("1f", "1b"), wh_sb, hcar, ccar, hs, o2f, xgd)

        # ---- xg2 (consumes hs1f/hs1b)
        def l2_rhs(k, n):
            src = hs["1f"] if k < 2 else hs["1b"]
            return src[:, n * 64:(n + 1) * 64, k % 2, :]

        with nc.named_scope("xg2"), \
             tc.tile_pool(name="xg2w", bufs=1) as xwp, \
             tc.tile_pool(name="xp2", bufs=4, space="PSUM") as xps, \
             tc.tile_pool(name="xs2", bufs=4) as xsb:
            for u in ("2f", "2b"):
                wx_sb = xwp.tile([128, KCNT[u] * MCNT[u] * 128], bf16,
                                 tag=f"wx{u}", name=f"wx2{u}")
                nc.sync.dma_start(wx_sb[:], wxd[u][:])
                wa_sb = xwp.tile([2, MCNT[u] * 128], bf16, tag=f"wa{u}",
                                 name=f"wa2{u}")
                nc.sync.dma_start(wa_sb[:], wad[u][:])
                for tI in range(NBODY):
                    xg_tile(nc, u, tI, l2_rhs, wx_sb, wa_sb, aug_sb, xgd[u],
                            xps, xsb)

        phase(nc, tc, ("2f", "2b"), wh_sb, hcar, ccar, hs, o2f, xgd)

        # ---- classifier
        with nc.named_scope("cls"), \
             tc.tile_pool(name="cl", bufs=3) as cp, \
             tc.tile_pool(name="cps", bufs=3, space="PSUM") as cps:
            for n in range(TB // 512):
                psm = cps.tile([TAGS, 512], f32, tag="ps")
                for k in range(4):
                    if k < 2:
                        rhs = o2f[:, 1 + n * 64:1 + (n + 1) * 64, k, :]
                    else:
                        rhs = hs["2b"][:, n * 64:(n + 1) * 64, k % 2, :]
                    nc.tensor.matmul(
                        out=psm[:],
                        lhsT=cls_sb[:, k * TAGS:(k + 1) * TAGS],
                        rhs=rhs,
                        start=(k == 0), stop=(k == 3))
                lg = cp.tile([TAGS, 512], f32, tag="lg")
                nc.vector.tensor_scalar_add(lg[:], psm[:], clsb_sb[:, :1])
                nc.sync.dma_start(logits_d[:, n * 512:(n + 1) * 512], lg[:])

    nc.compile()
    return nc


def xg_tile(nc, u, tI, rhs_of_k, wx_sb, wa_sb, aug_sb, xg_dram, xps, xsb):
    """One 512-token tile of the xg precompute (t in [64*tI, 64*tI+64))."""
    m_cnt, k_cnt = MCNT[u], KCNT[u]
    n = tI
    nsl = slice(n * 512, (n + 1) * 512)
    for m in range(m_cnt):
        psm = xps.tile([128, 512], f32, tag="ps")
        first = True
        if m < 8:  # gate chunks get the x contribution
            for k in range(k_cnt):
                nc.tensor.matmul(
                    out=psm[:],
                    lhsT=wx_sb[:, (k * m_cnt + m) * 128:(k * m_cnt + m + 1) * 128],
                    rhs=rhs_of_k(k, n),
                    start=first, stop=False)
                first = False
        nc.tensor.matmul(
            out=psm[:],
            lhsT=wa_sb[:, m * 128:(m + 1) * 128],
            rhs=aug_sb[:, nsl],
            start=first, stop=True)
        stg = xsb.tile([128, 512], bf16, tag="stg")
        if (n + m) % 2 == 0:
            nc.vector.tensor_copy(stg[:], psm[:])
        else:
            nc.scalar.activation(stg[:], psm[:],
                                 mybir.ActivationFunctionType.Copy)
        nc.sync.dma_start(
            xg_dram[:, m, n * 64:(n + 1) * 64, :],
            stg[:, :].rearrange("p (t b) -> p t b", b=Bc))


def phase(nc, tc, units, wh_sb, hcar, ccar, hs, o2f, xgd):
    """Static-unrolled recurrence for two direction units, step-interleaved."""
    MC = {u: 10 if u == "2f" else 8 for u in units}
    with nc.named_scope(f"ph{units[0]}"), \
         tc.tile_pool(name=f"rc{units[0]}", bufs=2) as rp, \
         tc.tile_pool(name=f"rps{units[0]}", bufs=2, space="PSUM") as rps, \
         tc.tile_pool(name=f"rtmp{units[0]}", bufs=4) as tp:

        def body_t0(u, i):
            return (NBODY - 1 - i) * SPB if REV[u] else i * SPB

        def dma_xb(u, i):
            xbt = rp.tile([128, MC[u], SPB, Bc], bf16, tag=f"xb{u}",
                          name=f"xb{u}")
            t0 = body_t0(u, i)
            nc.sync.dma_start(xbt[:, :, :, :],
                              xgd[u][:, 0:MC[u], t0:t0 + SPB, :])
            return xbt

        def prefill(psm, u, xbt, slot):
            nc.vector.tensor_copy(psm[:, :, :], xbt[:, 0:8, slot, :])

        xb = {u: dma_xb(u, 0) for u in units}
        xb_next = {}
        psum = {}
        for u in units:
            slot0 = (SPB - 1) if REV[u] else 0
            ps = rps.tile([128, 8, Bc], f32, tag=f"ps{u}", name=f"ps{u}")
            prefill(ps, u, xb[u], slot0)
            psum[u] = ps

        hstage_prev = {}
        for i in range(NBODY):
            hstage = {}
            for u in units:
                hstage[u] = rp.tile([128, SPB, 2, Bc], bf16, tag=f"hst{u}",
                                    name=f"hst{u}")
            for us in range(SPB):
                for u in units:
                    rev = REV[u]
                    slot = (SPB - 1 - us) if rev else us
                    if us == 0:
                        if i == 0:
                            hprev = hcar[u]
                        else:
                            pl = 0 if rev else SPB - 1
                            hprev = hstage_prev[u][:, pl, :, :]
                    else:
                        pslot = slot + 1 if rev else slot - 1
                        hprev = hstage[u][:, pslot, :, :]
                    psm = psum[u]
                    # 12 MMs for i,f,g chunks; sig(ifg) can fire while the
                    # 4 o-chunk MMs still run
                    for m in range(6):
                        for k in range(2):
                            nc.tensor.matmul(
                                out=psm[:, m, :],
                                lhsT=wh_sb[u][:, (k * 8 + m) * 128:(k * 8 + m + 1) * 128],
                                rhs=hprev[:, k, :],
                                start=False, stop=(k == 1),
                                skip_group_check=True)
                    for m in range(6, 8):
                        for k in range(2):
                            nc.tensor.matmul(
                                out=psm[:, m, :],
                                lhsT=wh_sb[u][:, (k * 8 + m) * 128:(k * 8 + m + 1) * 128],
                                rhs=hprev[:, k, :],
                                start=False, stop=(k == 1),
                                skip_group_check=True)
                    # prefetch next body's xb a bit into this body
                    if us == 4 and u == units[0] and i + 1 < NBODY:
                        for uu in units:
                            xb_next[uu] = dma_xb(uu, i + 1)
                    # prefill the NEXT step's PSUM (h-independent)
                    nxt_i = False
                    if us + 1 < SPB:
                        nslot = (slot - 1) if rev else (slot + 1)
                        nxb = xb[u]
                    elif i + 1 < NBODY:
                        nslot = (SPB - 1) if rev else 0
                        nxb = xb_next[u]
                        nxt_i = True
                    else:
                        nxb = None
                    if nxb is not None:
                        ps = rps.tile([128, 8, Bc], f32, tag=f"ps{u}",
                                      name=f"ps{u}")
                        prefill(ps, u, nxb, nslot)
                        psum[u] = ps
                    # pointwise tail (all-sigmoid)
                    sg = tp.tile([128, 8, Bc], f32, tag=f"sg{u}", name=f"sg{u}")
                    nc.scalar.activation(sg[:, 0:6, :], psm[:, 0:6, :], SIG)
                    nc.scalar.activation(sg[:, 6:8, :], psm[:, 6:8, :], SIG)
                    u1 = tp.tile([128, 2, Bc], f32, tag=f"u1{u}", name=f"u1{u}")
                    nc.vector.scalar_tensor_tensor(
                        out=u1[:, :, :], in0=sg[:, 4:6, :], scalar=-0.5,
                        in1=sg[:, 0:2, :], op0=ADD, op1=MUL)
                    csf = tp.tile([128, 2, Bc], f32, tag=f"csf{u}", name=f"csf{u}")
                    nc.vector.tensor_tensor(out=csf[:, :, :], in0=sg[:, 2:4, :],
                                            in1=ccar[u][:, :, :], op=MUL)
                    nc.vector.tensor_tensor(out=ccar[u][:, :, :],
                                            in0=csf[:, :, :], in1=u1[:, :, :],
                                            op=ADD)
                    sc = tp.tile([128, 2, Bc], f32, tag=f"sc{u}", name=f"sc{u}")
                    nc.scalar.activation(sc[:, :, :], ccar[u][:, :, :],
                                         SIG, scale=4.0)
                    nc.vector.scalar_tensor_tensor(
                        out=hstage[u][:, slot, :, :], in0=sc[:, :, :],
                        scalar=-0.5, in1=sg[:, 6:8, :], op0=ADD, op1=MUL)
            # ---- body end: flush history; 2f: scan-hold
            for u in units:
                rev = REV[u]
                t0 = body_t0(u, i)
                if u != "2f":
                    nc.gpsimd.tensor_copy(hs[u][:, t0:t0 + SPB, :, :],
                                          hstage[u][:, :, :, :])
                if u == "2f":
                    # held[t] = (1-m)[t]*held[t-1] + m[t]*h[t] along t
                    for k in range(2):
                        tmp = tp.tile([128, SPB, Bc], bf16, tag="tmp", name="tmp")
                        nc.vector.tensor_tensor(out=tmp[:, :, :],
                                                in0=hstage[u][:, :, k, :],
                                                in1=xb[u][:, 8, :, :], op=MUL)
                        for b in range(Bc):
                            nc.vector.tensor_tensor_scan(
                                out=o2f[:, 1 + t0:1 + t0 + SPB, k, b],
                                data0=xb[u][:, 9, :, b],
                                data1=tmp[:, :, b],
                                initial=o2f[:, t0, k, b:b + 1],
                                op0=MUL, op1=ADD)
            if i + 1 < NBODY:
                xb = dict(xb_next)
            hstage_prev = hstage


def _make_in_maps(inputs):
    words = np.asarray(inputs["words"]).astype(np.int32)
    lengths = np.asarray(inputs["lengths"]).astype(np.int32)
    emb = np.asarray(inputs["emb"], dtype=np.float32)
    mask = (lengths[:, None] > np.arange(T)[None, :]).astype(np.float32)
    wprep = {u: _prep_unit_weights(inputs[f"l{u}_Wih"], inputs[f"l{u}_Whh"],
                                   inputs[f"l{u}_bih"], inputs[f"l{u}_bhh"],
                                   MCNT[u], 2.0 if u[0] == "2" else 1.0)
             for u in UNITS}
    clsW = np.asarray(inputs["cls_W"], np.float64) * 2.0
    CT = clsW.T
    clsx = np.concatenate([CT[k * 128:(k + 1) * 128, :] for k in range(4)],
                          axis=1).astype(ml_dtypes.bfloat16)
    clsb = np.asarray(inputs["cls_b"], dtype=np.float32).reshape(TAGS, 1)
    in_maps = []
    for c in range(NCORES):
        bsl = slice(c * Bc, (c + 1) * Bc)
        w_c = words[bsl]
        m_c = mask[bsl]
        words_tm = np.ascontiguousarray(w_c.T).reshape(TB, 1)
        aug = np.stack([(1.0 - m_c.T).reshape(TB), np.ones(TB, np.float32)]
                       ).astype(ml_dtypes.bfloat16)
        im = {"emb": emb, "words": words_tm, "aug": aug,
              "clsx": clsx, "clsb": clsb}
        for u in UNITS:
            wx, wa, wh = wprep[u]
            im[f"w{u}x"] = wx
            im[f"w{u}a"] = wa
            im[f"w{u}h"] = wh
        in_maps.append(im)
    return in_maps


def kernel(**inputs):
    if "nc" not in _CACHE:
        _CACHE["nc"] = _build_program()
    nc = _CACHE["nc"]
    in_maps = _make_in_maps(inputs)
    _CACHE["in_maps"] = in_maps
    res = run_bass_kernel_spmd(nc, in_maps, list(range(NCORES)))
    out = np.empty((B, T, TAGS), np.float32)
    for c in range(NCORES):
        lg = res.results[c]["logits"]          # [50, TB], col = t*Bc + b
        out[c * Bc:(c + 1) * Bc] = lg.reshape(TAGS, T, Bc).transpose(2, 1, 0)
    return out


def bench(inputs):
    """Run once with NTFF tracing; returns HW exec_time_ns (and stashes trace)."""
    kernel(**inputs)  # ensure program built/cached
    nc = _CACHE["nc"]
    in_maps = _CACHE["in_maps"]
    import tempfile
    tmpdir = tempfile.mkdtemp(prefix="bilstm_trace_")
    res = run_bass_kernel_spmd(nc, in_maps, list(range(NCORES)), trace=True,
                               tmpdir=tmpdir)
    _CACHE["trace_dir"] = tmpdir
    _CACHE["last_bench"] = res
    print("trace dir:", tmpdir)
    if res.per_core_scope_times:
        for scope, times in res.per_core_scope_times.items():
            print(f"scope {scope}: {times}")
    return res.exec_time_ns


if __name__ == "__main__":
    import reference
    inputs = {k: np.asarray(v) for k, v in reference.setup_inputs().items()}
    got = kernel(**inputs)
    print(got.shape, got.dtype)


# revision 12
# speedup vs baseline: 1.2540x; 1.0547x over previous
"""BiLSTM tagger on 8 TRN2 NeuronCores.

Strategy (hardcoded for B=64,T=512,V=30000,E=128,H=256,TAGS=50):
  - Data-parallel: batch sharded 8 ways (8 sequences/core); weights replicated.
  - Per core: embedding gather (indirect DMA) -> PE transpose -> x^T in SBUF;
    input projections xg = W_ih_aug @ [x; 1-m; 1] precomputed for all t as big
    matmuls into DRAM scratch (middle-out tile order so ph1 can start after
    2 tiles); recurrences fully STATICALLY UNROLLED (no dynamic loops), xb
    DMAs prefetched one 64-step body ahead; classifier.
  - ALL-SIGMOID formulation: tanh(x) = 2*sigmoid(2x)-1 folded into weight
    scaling. Carried state is h/2, c/2; consumers' weights pre-scaled by 2
    (Whh, l2 Wih, cls_W), g-gate rows by an extra 2. Every activation is
    Sigmoid (tanh(c) = sig(4*(c/2)) via the activation scale arg) -> zero
    act-table switches.
  - Per unit-step: DVE prefills PSUM with xg (h-independent, off critical
    path); 12 Whh matmuls for i,f,g chunks then sig(ifg) fires while the 4
    o-chunk matmuls still run; DVE u1=(sg-.5)*si, csf=sf*c', c'=csf+u1;
    Act sig(4c'); DVE h'=(sc-.5)*so -> staged bf16.
  - Masking: +/-60*(1-m) on i/f gate pre-activations freezes c exactly at
    masked steps; backward h is exactly 0 there. The l2f held output is
    reconstructed per-body with tensor_tensor_scan along t:
    held[t] = (1-m[t])*held[t-1] + m[t]*h[t].
"""
import sys

sys.path.insert(0, "/opt/trn_rl_repo")
import contextlib

import numpy as np
import ml_dtypes

import concourse.bass as bass
import concourse.bacc as bacc
import concourse.mybir as mybir
import concourse.tile as tile
from concourse.bass_utils import run_bass_kernel_spmd
from concourse.masks import make_identity

B, T, V, E, H, TAGS = 64, 512, 30000, 128, 256, 50
NCORES = 8
Bc = B // NCORES          # 8 sequences per core
TB = T * Bc               # 4096 tokens per core
SPB = 64                  # steps per body
NBODY = T // SPB          # 8

f32 = mybir.dt.float32
bf16 = mybir.dt.bfloat16
i32 = mybir.dt.int32

UNITS = ("1f", "1b", "2f", "2b")
KCNT = {"1f": 1, "1b": 1, "2f": 4, "2b": 4}       # 128-row K chunks of x features
MCNT = {"1f": 8, "1b": 8, "2f": 10, "2b": 8}      # 128-row output chunks
REV = {"1f": False, "1b": True, "2f": False, "2b": True}

_CACHE = {}

SIG = mybir.ActivationFunctionType.Sigmoid
ADD = mybir.AluOpType.add
MUL = mybir.AluOpType.mult

# gate-row order [i(0:256), f(256:512), g(512:768), o(768:1024)] = torch order
# with g and o swapped
PERM = np.concatenate([np.arange(0, 512), np.arange(512, 768),
                       np.arange(768, 1024)])  # identity on i,f; then g; then o


def _prep_unit_weights(Wih, Whh, bih, bhh, m_cnt, in_scale):
    """Host-side weight marshalling (all-sigmoid form).

    Torch row order is [i f g o]; we keep it (i=chunks0:2, f=2:4, g=4:6,
    o=6:8). in_scale compensates h/2-scaled inputs (2.0 for l2). Whh x2
    (recurrent h is h/2); g rows an extra x2 (tanh = 2*sig(2x)-1)."""
    din = Wih.shape[1]
    Wp = np.asarray(Wih, np.float64) * in_scale   # [1024, din]
    Up = np.asarray(Whh, np.float64) * 2.0        # [1024, 256]
    bp = (np.asarray(bih, np.float64) + np.asarray(bhh, np.float64)).copy()
    Wp = Wp.copy()
    Wp[512:768] *= 2.0
    Up = Up.copy()
    Up[512:768] *= 2.0
    bp[512:768] *= 2.0
    M = m_cnt * 128
    k_cnt = din // 128
    # x-part lhsT: [din, M] -> k-chunk-major cols [128, k_cnt*M]
    WT = np.zeros((din, M), np.float64)
    WT[:, :1024] = Wp.T
    wx = np.concatenate([WT[k * 128:(k + 1) * 128, :] for k in range(k_cnt)],
                        axis=1).astype(ml_dtypes.bfloat16)  # [128, k_cnt*M]
    # aug lhsT rows: feature0 = (1-m), feature1 = 1
    wa = np.zeros((2, M), np.float64)
    wa[0, 0:256] = -60.0   # i rows: -60*(1-m)
    wa[0, 256:512] = 60.0  # f rows: +60*(1-m)
    wa[1, :1024] = bp
    if m_cnt == 10:        # l2f extra planes: chunk8 = m, chunk9 = 1-m
        wa[0, 1024:1152] = -1.0
        wa[1, 1024:1152] = 1.0
        wa[0, 1152:1280] = 1.0
        wa[1, 1152:1280] = 0.0
    wa = wa.astype(ml_dtypes.bfloat16)
    # Whh lhsT: [256, 1024] -> [128, 2*1024], (k*8+m) chunk indexing
    UT = Up.T
    wh = np.concatenate([UT[0:128, :], UT[128:256, :]], axis=1).astype(ml_dtypes.bfloat16)
    return wx, wa, wh


def _build_program():
    nc = bacc.Bacc("TRN2", target_bir_lowering=False, debug=False, num_devices=NCORES)
    emb_d = nc.dram_tensor("emb", [V, E], f32, kind="ExternalInput")
    words_d = nc.dram_tensor("words", [TB, 1], i32, kind="ExternalInput")
    aug_d = nc.dram_tensor("aug", [2, TB], bf16, kind="ExternalInput")
    wxd, wad, whd, xgd = {}, {}, {}, {}
    for u in UNITS:
        wxd[u] = nc.dram_tensor(f"w{u}x", [128, KCNT[u] * MCNT[u] * 128], bf16, kind="ExternalInput")
        wad[u] = nc.dram_tensor(f"w{u}a", [2, MCNT[u] * 128], bf16, kind="ExternalInput")
        whd[u] = nc.dram_tensor(f"w{u}h", [128, 2048], bf16, kind="ExternalInput")
        xgd[u] = [nc.dram_tensor(f"xg{u}t{t}", [128, MCNT[u], SPB, Bc], bf16)
                  for t in range(NBODY)]
    clsx_d = nc.dram_tensor("clsx", [128, 4 * TAGS], bf16, kind="ExternalInput")
    clsb_d = nc.dram_tensor("clsb", [TAGS, 1], f32, kind="ExternalInput")
    logits_d = nc.dram_tensor("logits", [TAGS, TB], f32, kind="ExternalOutput")

    ctx = contextlib.ExitStack()
    with tile.TileContext(nc) as tc, ctx:
        pp = ctx.enter_context(tc.tile_pool(name="persist", bufs=1))
        aug_sb = pp.tile([2, TB], bf16, tag="aug")
        wh_sb = {u: pp.tile([128, 2048], bf16, tag=f"wh{u}", name=f"wh{u}") for u in UNITS}
        cls_sb = pp.tile([128, 4 * TAGS], bf16, tag="clsx")
        clsb_sb = pp.tile([TAGS, 1], f32, tag="clsb")
        hs = {u: pp.tile([128, T, 2, Bc], bf16, tag=f"hs{u}", name=f"hs{u}")
              for u in ("1f", "1b", "2b")}
        o2f = pp.tile([128, T + 1, 2, Bc], bf16, tag="o2f")  # col0 = zeros
        identb = pp.tile([128, 128], bf16, tag="identb")
        hcar = {u: pp.tile([128, 2, Bc], bf16, tag=f"hc{u}", name=f"hc{u}") for u in UNITS}
        ccar = {u: pp.tile([128, 2, Bc], f32, tag=f"cc{u}", name=f"cc{u}") for u in UNITS}

        for u in UNITS:
            nc.sync.dma_start(wh_sb[u][:], whd[u][:])
        nc.sync.dma_start(cls_sb[:], clsx_d[:])
        nc.sync.dma_start(clsb_sb[:], clsb_d[:])
        nc.sync.dma_start(aug_sb[:], aug_d[:])
        make_identity(nc, identb[:])
        for u in UNITS:
            nc.vector.memset(hcar[u][:, :, :], 0.0)
            nc.vector.memset(ccar[u][:, :, :], 0.0)
        nc.vector.memset(o2f[:, 0, :, :], 0.0)

        # ---- prologue: gather + xg1 interleaved, middle-out tile order
        with nc.named_scope("gather"), \
             tc.tile_pool(name="prolog", bufs=1) as lp, \
             tc.tile_pool(name="gat", bufs=4) as gp, \
             tc.tile_pool(name="gps", bufs=4, space="PSUM") as gps, \
             tc.tile_pool(name="xp1", bufs=4, space="PSUM") as xps, \
             tc.tile_pool(name="xs1", bufs=2) as xsb:
            xT = lp.tile([128, TB], bf16, tag="xT")
            ident = lp.tile([128, 128], f32, tag="ident")
            make_identity(nc, ident[:])
            wx1, wa1 = {}, {}
            for u in ("1f", "1b"):
                wx1[u] = lp.tile([128, KCNT[u] * MCNT[u] * 128], bf16,
                                 tag=f"wx{u}", name=f"wx{u}")
                nc.sync.dma_start(wx1[u][:], wxd[u][:])
                wa1[u] = lp.tile([2, MCNT[u] * 128], bf16, tag=f"wa{u}",
                                 name=f"wa{u}")
                nc.sync.dma_start(wa1[u][:], wad[u][:])

            def l1_rhs(k, n):
                return xT[:, n * 512:(n + 1) * 512]

            order = [0, 7, 1, 6, 2, 5, 3, 4]
            for tI in order:
                for j in range(4):
                    n = tI * 4 + j
                    idx = gp.tile([128, 1], i32, tag="idx")
                    nc.sync.dma_start(idx[:], words_d[n * 128:(n + 1) * 128, :])
                    xt = gp.tile([128, 128], f32, tag="xt")
                    nc.gpsimd.indirect_dma_start(
                        out=xt[:], out_offset=None, in_=emb_d[:, :],
                        in_offset=bass.IndirectOffsetOnAxis(ap=idx[:, :1], axis=0))
                    pst = gps.tile([128, 128], f32, tag="pst")
                    nc.tensor.transpose(out=pst[:], in_=xt[:], identity=ident[:])
                    nc.vector.tensor_copy(xT[:, n * 128:(n + 1) * 128], pst[:])
                for u in ("1f", "1b"):
                    xg_tile(nc, u, tI, l1_rhs, wx1[u], wa1[u], aug_sb, xgd[u],
                            xps, xsb)

        phase(nc, tc, ("1f", "1b"), wh_sb, hcar, ccar, hs, o2f, xgd, identb)

        # ---- xg2 (consumes hs1f/hs1b)
        def l2_rhs(k, n):
            src = hs["1f"] if k < 2 else hs["1b"]
            return src[:, n * 64:(n + 1) * 64, k % 2, :]

        with nc.named_scope("xg2"), \
             tc.tile_pool(name="xg2w", bufs=1) as xwp, \
             tc.tile_pool(name="xp2", bufs=4, space="PSUM") as xps, \
             tc.tile_pool(name="xs2", bufs=2) as xsb:
            wx2, wa2 = {}, {}
            for u in ("2f", "2b"):
                wx2[u] = xwp.tile([128, KCNT[u] * MCNT[u] * 128], bf16,
                                  tag=f"wx{u}", name=f"wx2{u}")
                nc.sync.dma_start(wx2[u][:], wxd[u][:])
                wa2[u] = xwp.tile([2, MCNT[u] * 128], bf16, tag=f"wa{u}",
                                  name=f"wa2{u}")
                nc.sync.dma_start(wa2[u][:], wad[u][:])

            def xg2_pair(p):
                # pair p = {2f tile p, 2b tile 7-p}; ph2 body i consumes
                # 2f tile i and 2b tile 7-i, i.e. exactly pair i.
                xg_tile(nc, "2f", p, l2_rhs, wx2["2f"], wa2["2f"], aug_sb,
                        xgd["2f"], xps, xsb)
                xg_tile(nc, "2b", 7 - p, l2_rhs, wx2["2b"], wa2["2b"], aug_sb,
                        xgd["2b"], xps, xsb)

            xg2_pair(0)
            xg2_pair(1)

            def pre_body2(i):
                if i + 2 < NBODY:
                    xg2_pair(i + 2)

            phase(nc, tc, ("2f", "2b"), wh_sb, hcar, ccar, hs, o2f, xgd,
                  identb, pre_body=pre_body2)

        # ---- classifier
        with nc.named_scope("cls"), \
             tc.tile_pool(name="cl", bufs=3) as cp, \
             tc.tile_pool(name="cps", bufs=3, space="PSUM") as cps:
            for n in range(TB // 512):
                psm = cps.tile([TAGS, 512], f32, tag="ps")
                for k in range(4):
                    if k < 2:
                        rhs = o2f[:, 1 + n * 64:1 + (n + 1) * 64, k, :]
                    else:
                        rhs = hs["2b"][:, n * 64:(n + 1) * 64, k % 2, :]
                    nc.tensor.matmul(
                        out=psm[:],
                        lhsT=cls_sb[:, k * TAGS:(k + 1) * TAGS],
                        rhs=rhs,
                        start=(k == 0), stop=(k == 3))
                lg = cp.tile([TAGS, 512], f32, tag="lg")
                nc.vector.tensor_scalar_add(lg[:], psm[:], clsb_sb[:, :1])
                nc.sync.dma_start(logits_d[:, n * 512:(n + 1) * 512], lg[:])

    nc.compile()
    return nc


def xg_tile(nc, u, tI, rhs_of_k, wx_sb, wa_sb, aug_sb, xg_dram, xps, xsb):
    """One 512-token tile of the xg precompute (t in [64*tI, 64*tI+64))."""
    m_cnt, k_cnt = MCNT[u], KCNT[u]
    n = tI
    nsl = slice(n * 512, (n + 1) * 512)
    stg = xsb.tile([128, m_cnt, 512], bf16, tag=f"stg{u}", name=f"stg{u}")
    for m in range(m_cnt):
        psm = xps.tile([128, 512], f32, tag="ps")
        first = True
        if m < 8:  # gate chunks get the x contribution
            for k in range(k_cnt):
                nc.tensor.matmul(
                    out=psm[:],
                    lhsT=wx_sb[:, (k * m_cnt + m) * 128:(k * m_cnt + m + 1) * 128],
                    rhs=rhs_of_k(k, n),
                    start=first, stop=False)
                first = False
        nc.tensor.matmul(
            out=psm[:],
            lhsT=wa_sb[:, m * 128:(m + 1) * 128],
            rhs=aug_sb[:, nsl],
            start=first, stop=True)
        if (n + m) % 2 == 0:
            nc.vector.tensor_copy(stg[:, m, :], psm[:])
        else:
            nc.scalar.activation(stg[:, m, :], psm[:],
                                 mybir.ActivationFunctionType.Copy)
    nc.sync.dma_start(
        xg_dram[tI][:, :, :, :],
        stg[:, :, :].rearrange("p m (t b) -> p m t b", b=Bc))


def phase(nc, tc, units, wh_sb, hcar, ccar, hs, o2f, xgd, identb,
          pre_body=None):
    """Static-unrolled recurrence for two direction units, step-interleaved."""
    MC = {u: 10 if u == "2f" else 8 for u in units}
    with nc.named_scope(f"ph{units[0]}"), \
         tc.tile_pool(name=f"rc{units[0]}", bufs=2) as rp, \
         tc.tile_pool(name=f"rps{units[0]}", bufs=2, space="PSUM") as rps, \
         tc.tile_pool(name=f"rtmp{units[0]}", bufs=4) as tp, \
         tc.tile_pool(name=f"rsc{units[0]}", bufs=2) as scp:

        def body_t0(u, i):
            return (NBODY - 1 - i) * SPB if REV[u] else i * SPB

        def dma_xb(u, i):
            xbt = rp.tile([128, MC[u], SPB, Bc], bf16, tag=f"xb{u}",
                          name=f"xb{u}")
            tI = (NBODY - 1 - i) if REV[u] else i
            nc.sync.dma_start(xbt[:, :, :, :], xgd[u][tI][:, :, :, :])
            return xbt

        def prefill(psm, u, xbt, slot):
            nc.tensor.matmul(out=psm[:, :, :], lhsT=identb[:],
                             rhs=xbt[:, 0:8, slot, :],
                             start=True, stop=False, skip_group_check=True)

        xb = {u: dma_xb(u, 0) for u in units}
        xb_next = {}
        psum = {}
        for u in units:
            slot0 = (SPB - 1) if REV[u] else 0
            ps = rps.tile([128, 8, Bc], f32, tag=f"ps{u}", name=f"ps{u}")
            prefill(ps, u, xb[u], slot0)
            psum[u] = ps

        hstage_prev = {}
        deferred = []
        for i in range(NBODY):
            if pre_body is not None:
                pre_body(i)
            hstage = {}
            for u in units:
                hstage[u] = rp.tile([128, SPB, 2, Bc], bf16, tag=f"hst{u}",
                                    name=f"hst{u}")
            for us in range(SPB):
                for u in units:
                    rev = REV[u]
                    slot = (SPB - 1 - us) if rev else us
                    if us == 0:
                        if i == 0:
                            hprev = hcar[u]
                        else:
                            pl = 0 if rev else SPB - 1
                            hprev = hstage_prev[u][:, pl, :, :]
                    else:
                        pslot = slot + 1 if rev else slot - 1
                        hprev = hstage[u][:, pslot, :, :]
                    psm = psum[u]
                    # 12 MMs for i,f,g chunks; sig(ifg) can fire while the
                    # 4 o-chunk MMs still run
                    for m in range(6):
                        for k in range(2):
                            nc.tensor.matmul(
                                out=psm[:, m, :],
                                lhsT=wh_sb[u][:, (k * 8 + m) * 128:(k * 8 + m + 1) * 128],
                                rhs=hprev[:, k, :],
                                start=False, stop=(k == 1),
                                skip_group_check=True)
                    for m in range(6, 8):
                        for k in range(2):
                            nc.tensor.matmul(
                                out=psm[:, m, :],
                                lhsT=wh_sb[u][:, (k * 8 + m) * 128:(k * 8 + m + 1) * 128],
                                rhs=hprev[:, k, :],
                                start=False, stop=(k == 1),
                                skip_group_check=True)
                    # prefetch next body's xb a bit into this body
                    if us == 4 and u == units[0] and i + 1 < NBODY:
                        for uu in units:
                            xb_next[uu] = dma_xb(uu, i + 1)
                    # prefill the NEXT step's PSUM (h-independent)
                    nxt_i = False
                    if us + 1 < SPB:
                        nslot = (slot - 1) if rev else (slot + 1)
                        nxb = xb[u]
                    elif i + 1 < NBODY:
                        nslot = (SPB - 1) if rev else 0
                        nxb = xb_next[u]
                        nxt_i = True
                    else:
                        nxb = None
                    if nxb is not None:
                        ps = rps.tile([128, 8, Bc], f32, tag=f"ps{u}",
                                      name=f"ps{u}")
                        prefill(ps, u, nxb, nslot)
                        psum[u] = ps
                    # pointwise tail (all-sigmoid)
                    sg = tp.tile([128, 8, Bc], f32, tag=f"sg{u}", name=f"sg{u}")
                    nc.scalar.activation(sg[:, 0:6, :], psm[:, 0:6, :], SIG)
                    nc.scalar.activation(sg[:, 6:8, :], psm[:, 6:8, :], SIG)
                    u1 = tp.tile([128, 2, Bc], f32, tag=f"u1{u}", name=f"u1{u}")
                    nc.vector.scalar_tensor_tensor(
                        out=u1[:, :, :], in0=sg[:, 4:6, :], scalar=-0.5,
                        in1=sg[:, 0:2, :], op0=ADD, op1=MUL)
                    csf = tp.tile([128, 2, Bc], f32, tag=f"csf{u}", name=f"csf{u}")
                    nc.vector.tensor_tensor(out=csf[:, :, :], in0=sg[:, 2:4, :],
                                            in1=ccar[u][:, :, :], op=MUL)
                    nc.vector.tensor_tensor(out=ccar[u][:, :, :],
                                            in0=csf[:, :, :], in1=u1[:, :, :],
                                            op=ADD)
                    sc = tp.tile([128, 2, Bc], f32, tag=f"sc{u}", name=f"sc{u}")
                    nc.scalar.activation(sc[:, :, :], ccar[u][:, :, :],
                                         SIG, scale=4.0)
                    nc.vector.scalar_tensor_tensor(
                        out=hstage[u][:, slot, :, :], in0=sc[:, :, :],
                        scalar=-0.5, in1=sg[:, 6:8, :], op0=ADD, op1=MUL)
                if deferred and us % 4 == 3:
                    deferred.pop(0)()
            # ---- body end: flush history; 2f: scan-hold
            for u in units:
                rev = REV[u]
                t0 = body_t0(u, i)
                if u != "2f":
                    nc.gpsimd.tensor_copy(hs[u][:, t0:t0 + SPB, :, :],
                                          hstage[u][:, :, :, :])
                if u == "2f":
                    # held[t] = (1-m)[t]*held[t-1] + m[t]*h[t] along t.
                    # Planes are snapshotted (GpSimd) to decouple from the xb
                    # ring; the scans themselves are DEFERRED and spread over
                    # the next body's step slots to keep the DVE queue clear.
                    mpl = scp.tile([128, 2, SPB, Bc], bf16, tag="mpl", name="mpl")
                    nc.gpsimd.tensor_copy(mpl[:, :, :, :], xb[u][:, 8:10, :, :])
                    hst2f = hstage[u]
                    for k in range(2):
                        tmp = scp.tile([128, SPB, Bc], bf16, tag=f"tmp{k}",
                                      name=f"tmp{k}")
                        nc.gpsimd.tensor_tensor(out=tmp[:, :, :],
                                                in0=hst2f[:, :, k, :],
                                                in1=mpl[:, 0, :, :], op=MUL)
                        for b in range(Bc):
                            def mk_scan(k=k, b=b, t0=t0, tmp=tmp, mpl=mpl):
                                nc.vector.tensor_tensor_scan(
                                    out=o2f[:, 1 + t0:1 + t0 + SPB, k, b],
                                    data0=mpl[:, 1, :, b],
                                    data1=tmp[:, :, b],
                                    initial=o2f[:, t0, k, b:b + 1],
                                    op0=MUL, op1=ADD)
                            deferred.append(mk_scan)
            if i + 1 < NBODY:
                xb = dict(xb_next)
            hstage_prev = hstage
        for fn in deferred:
            fn()


def _make_in_maps(inputs):
    words = np.asarray(inputs["words"]).astype(np.int32)
    lengths = np.asarray(inputs["lengths"]).astype(np.int32)
    emb = np.asarray(inputs["emb"], dtype=np.float32)
    mask = (lengths[:, None] > np.arange(T)[None, :]).astype(np.float32)
    wprep = {u: _prep_unit_weights(inputs[f"l{u}_Wih"], inputs[f"l{u}_Whh"],
                                   inputs[f"l{u}_bih"], inputs[f"l{u}_bhh"],
                                   MCNT[u], 2.0 if u[0] == "2" else 1.0)
             for u in UNITS}
    clsW = np.asarray(inputs["cls_W"], np.float64) * 2.0
    CT = clsW.T
    clsx = np.concatenate([CT[k * 128:(k + 1) * 128, :] for k in range(4)],
                          axis=1).astype(ml_dtypes.bfloat16)
    clsb = np.asarray(inputs["cls_b"], dtype=np.float32).reshape(TAGS, 1)
    in_maps = []
    for c in range(NCORES):
        bsl = slice(c * Bc, (c + 1) * Bc)
        w_c = words[bsl]
        m_c = mask[bsl]
        words_tm = np.ascontiguousarray(w_c.T).reshape(TB, 1)
        aug = np.stack([(1.0 - m_c.T).reshape(TB), np.ones(TB, np.float32)]
                       ).astype(ml_dtypes.bfloat16)
        im = {"emb": emb, "words": words_tm, "aug": aug,
              "clsx": clsx, "clsb": clsb}
        for u in UNITS:
            wx, wa, wh = wprep[u]
            im[f"w{u}x"] = wx
            im[f"w{u}a"] = wa
            im[f"w{u}h"] = wh
        in_maps.append(im)
    return in_maps


def kernel(**inputs):
    if "nc" not in _CACHE:
        _CACHE["nc"] = _build_program()
    nc = _CACHE["nc"]
    in_maps = _make_in_maps(inputs)
    _CACHE["in_maps"] = in_maps
    res = run_bass_kernel_spmd(nc, in_maps, list(range(NCORES)))
    out = np.empty((B, T, TAGS), np.float32)
    for c in range(NCORES):
        lg = res.results[c]["logits"]          # [50, TB], col = t*Bc + b
        out[c * Bc:(c + 1) * Bc] = lg.reshape(TAGS, T, Bc).transpose(2, 1, 0)
    return out


def bench(inputs):
    """Run once with NTFF tracing; returns HW exec_time_ns (and stashes trace)."""
    kernel(**inputs)  # ensure program built/cached
    nc = _CACHE["nc"]
    in_maps = _CACHE["in_maps"]
    import tempfile
    tmpdir = tempfile.mkdtemp(prefix="bilstm_trace_")
    res = run_bass_kernel_spmd(nc, in_maps, list(range(NCORES)), trace=True,
                               tmpdir=tmpdir)
    _CACHE["trace_dir"] = tmpdir
    _CACHE["last_bench"] = res
    print("trace dir:", tmpdir)
    if res.per_core_scope_times:
        for scope, times in res.per_core_scope_times.items():
            print(f"scope {scope}: {times}")
    return res.exec_time_ns


if __name__ == "__main__":
    import reference
    inputs = {k: np.asarray(v) for k, v in reference.setup_inputs().items()}
    got = kernel(**inputs)
    print(got.shape, got.dtype)


# revision 14
# speedup vs baseline: 1.3268x; 1.0580x over previous
"""BiLSTM tagger on 8 TRN2 NeuronCores.

Strategy (hardcoded for B=64,T=512,V=30000,E=128,H=256,TAGS=50):
  - Data-parallel: batch sharded 8 ways (8 sequences/core); weights replicated.
  - Per core: embedding gather (indirect DMA) -> PE transpose -> x^T in SBUF;
    input projections xg = W_ih_aug @ [x; 1-m; 1] precomputed for all t as big
    matmuls into DRAM scratch (middle-out tile order so ph1 can start after
    2 tiles); recurrences fully STATICALLY UNROLLED (no dynamic loops), xb
    DMAs prefetched one 64-step body ahead; classifier.
  - ALL-SIGMOID formulation: tanh(x) = 2*sigmoid(2x)-1 folded into weight
    scaling. Carried state is h/2, c/2; consumers' weights pre-scaled by 2
    (Whh, l2 Wih, cls_W), g-gate rows by an extra 2. Every activation is
    Sigmoid (tanh(c) = sig(4*(c/2)) via the activation scale arg) -> zero
    act-table switches.
  - Per unit-step: DVE prefills PSUM with xg (h-independent, off critical
    path); 12 Whh matmuls for i,f,g chunks then sig(ifg) fires while the 4
    o-chunk matmuls still run; DVE u1=(sg-.5)*si, csf=sf*c', c'=csf+u1;
    Act sig(4c'); DVE h'=(sc-.5)*so -> staged bf16.
  - Masking: +/-60*(1-m) on i/f gate pre-activations freezes c exactly at
    masked steps; backward h is exactly 0 there. The l2f held output is
    reconstructed per-body with tensor_tensor_scan along t:
    held[t] = (1-m[t])*held[t-1] + m[t]*h[t].
"""
import sys

sys.path.insert(0, "/opt/trn_rl_repo")
import contextlib

import numpy as np
import ml_dtypes

import concourse.bass as bass
import concourse.bacc as bacc
import concourse.mybir as mybir
import concourse.tile as tile
from concourse.bass_utils import run_bass_kernel_spmd
from concourse.masks import make_identity

B, T, V, E, H, TAGS = 64, 512, 30000, 128, 256, 50
NCORES = 8
Bc = B // NCORES          # 8 sequences per core
TB = T * Bc               # 4096 tokens per core
SPB = 64                  # steps per body
NBODY = T // SPB          # 8

f32 = mybir.dt.float32
bf16 = mybir.dt.bfloat16
i32 = mybir.dt.int32

UNITS = ("1f", "1b", "2f", "2b")
KCNT = {"1f": 1, "1b": 1, "2f": 4, "2b": 4}       # 128-row K chunks of x features
MCNT = {"1f": 8, "1b": 8, "2f": 10, "2b": 8}      # 128-row output chunks
REV = {"1f": False, "1b": True, "2f": False, "2b": True}

_CACHE = {}

SIG = mybir.ActivationFunctionType.Sigmoid
ADD = mybir.AluOpType.add
MUL = mybir.AluOpType.mult

# gate-row order [i(0:256), f(256:512), g(512:768), o(768:1024)] = torch order
# with g and o swapped
PERM = np.concatenate([np.arange(0, 512), np.arange(512, 768),
                       np.arange(768, 1024)])  # identity on i,f; then g; then o


def _prep_unit_weights(Wih, Whh, bih, bhh, m_cnt, in_scale):
    """Host-side weight marshalling (all-sigmoid form).

    Torch row order is [i f g o]; we keep it (i=chunks0:2, f=2:4, g=4:6,
    o=6:8). in_scale compensates h/2-scaled inputs (2.0 for l2). Whh x2
    (recurrent h is h/2); g rows an extra x2 (tanh = 2*sig(2x)-1)."""
    din = Wih.shape[1]
    Wp = np.asarray(Wih, np.float64) * in_scale   # [1024, din]
    Up = np.asarray(Whh, np.float64) * 2.0        # [1024, 256]
    bp = (np.asarray(bih, np.float64) + np.asarray(bhh, np.float64)).copy()
    Wp = Wp.copy()
    Wp[512:768] *= 2.0
    Up = Up.copy()
    Up[512:768] *= 2.0
    bp[512:768] *= 2.0
    M = m_cnt * 128
    k_cnt = din // 128
    # x-part lhsT: [din, M] -> k-chunk-major cols [128, k_cnt*M]
    WT = np.zeros((din, M), np.float64)
    WT[:, :1024] = Wp.T
    wx = np.concatenate([WT[k * 128:(k + 1) * 128, :] for k in range(k_cnt)],
                        axis=1).astype(ml_dtypes.bfloat16)  # [128, k_cnt*M]
    # aug lhsT rows: feature0 = (1-m), feature1 = 1
    wa = np.zeros((2, M), np.float64)
    wa[0, 0:256] = -60.0   # i rows: -60*(1-m)
    wa[0, 256:512] = 60.0  # f rows: +60*(1-m)
    wa[1, :1024] = bp
    if m_cnt == 10:        # l2f extra planes: chunk8 = m, chunk9 = 1-m
        wa[0, 1024:1152] = -1.0
        wa[1, 1024:1152] = 1.0
        wa[0, 1152:1280] = 1.0
        wa[1, 1152:1280] = 0.0
    wa = wa.astype(ml_dtypes.bfloat16)
    # Whh lhsT: [256, 1024] -> [128, 2*1024], (k*8+m) chunk indexing
    UT = Up.T
    wh = np.concatenate([UT[0:128, :], UT[128:256, :]], axis=1).astype(ml_dtypes.bfloat16)
    return wx, wa, wh


def _build_program():
    nc = bacc.Bacc("TRN2", target_bir_lowering=False, debug=False, num_devices=NCORES)
    emb_d = nc.dram_tensor("emb", [V, E], f32, kind="ExternalInput")
    words_d = nc.dram_tensor("words", [TB, 1], i32, kind="ExternalInput")
    aug_d = nc.dram_tensor("aug", [2, TB], bf16, kind="ExternalInput")
    wxd, wad, whd, xgd = {}, {}, {}, {}
    for u in UNITS:
        wxd[u] = nc.dram_tensor(f"w{u}x", [128, KCNT[u] * MCNT[u] * 128], bf16, kind="ExternalInput")
        wad[u] = nc.dram_tensor(f"w{u}a", [2, MCNT[u] * 128], bf16, kind="ExternalInput")
        whd[u] = nc.dram_tensor(f"w{u}h", [128, 2048], bf16, kind="ExternalInput")
        xgd[u] = [nc.dram_tensor(f"xg{u}t{t}", [128, MCNT[u], SPB, Bc], bf16)
                  for t in range(NBODY)]
    clsx_d = nc.dram_tensor("clsx", [128, 4 * TAGS], bf16, kind="ExternalInput")
    clsb_d = nc.dram_tensor("clsb", [TAGS, 1], f32, kind="ExternalInput")
    logits_d = nc.dram_tensor("logits", [TAGS, TB], f32, kind="ExternalOutput")

    ctx = contextlib.ExitStack()
    with tile.TileContext(nc) as tc, ctx:
        pp = ctx.enter_context(tc.tile_pool(name="persist", bufs=1))
        aug_sb = pp.tile([2, TB], bf16, tag="aug")
        wh_sb = {u: pp.tile([128, 2048], bf16, tag=f"wh{u}", name=f"wh{u}") for u in UNITS}
        cls_sb = pp.tile([128, 4 * TAGS], bf16, tag="clsx")
        clsb_sb = pp.tile([TAGS, 1], f32, tag="clsb")
        hs = {u: pp.tile([128, T, 2, Bc], bf16, tag=f"hs{u}", name=f"hs{u}")
              for u in ("1f", "1b", "2b")}
        o2f = pp.tile([128, T + 1, 2, Bc], bf16, tag="o2f")  # col0 = zeros
        identb = pp.tile([128, 128], bf16, tag="identb")
        hcar = {u: pp.tile([128, 2, Bc], bf16, tag=f"hc{u}", name=f"hc{u}") for u in UNITS}
        ccar = {u: pp.tile([128, 2, Bc], f32, tag=f"cc{u}", name=f"cc{u}") for u in UNITS}

        for u in UNITS:
            nc.sync.dma_start(wh_sb[u][:], whd[u][:])
        nc.sync.dma_start(cls_sb[:], clsx_d[:])
        nc.sync.dma_start(clsb_sb[:], clsb_d[:])
        nc.sync.dma_start(aug_sb[:], aug_d[:])
        make_identity(nc, identb[:])
        for u in UNITS:
            nc.vector.memset(hcar[u][:, :, :], 0.0)
            nc.vector.memset(ccar[u][:, :, :], 0.0)
        nc.vector.memset(o2f[:, 0, :, :], 0.0)

        # ---- prologue: gather + xg1 interleaved, middle-out tile order
        with nc.named_scope("gather"), \
             tc.tile_pool(name="prolog", bufs=1) as lp, \
             tc.tile_pool(name="gat", bufs=4) as gp, \
             tc.tile_pool(name="gps", bufs=2, space="PSUM") as gps, \
             tc.tile_pool(name="xp1", bufs=2, space="PSUM") as xps, \
             tc.tile_pool(name="xs1", bufs=2) as xsb:
            xT = lp.tile([128, TB], bf16, tag="xT")
            ident = lp.tile([128, 128], f32, tag="ident")
            make_identity(nc, ident[:])
            wx1, wa1 = {}, {}
            for u in ("1f", "1b"):
                wx1[u] = lp.tile([128, KCNT[u] * MCNT[u] * 128], bf16,
                                 tag=f"wx{u}", name=f"wx{u}")
                nc.sync.dma_start(wx1[u][:], wxd[u][:])
                wa1[u] = lp.tile([2, MCNT[u] * 128], bf16, tag=f"wa{u}",
                                 name=f"wa{u}")
                nc.sync.dma_start(wa1[u][:], wad[u][:])

            def l1_rhs(k, n):
                return xT[:, n * 512:(n + 1) * 512]

            def gather_tile(tI):
                for j in range(4):
                    n = tI * 4 + j
                    idx = gp.tile([128, 1], i32, tag="idx")
                    nc.sync.dma_start(idx[:], words_d[n * 128:(n + 1) * 128, :])
                    xt = gp.tile([128, 128], f32, tag="xt")
                    nc.gpsimd.indirect_dma_start(
                        out=xt[:], out_offset=None, in_=emb_d[:, :],
                        in_offset=bass.IndirectOffsetOnAxis(ap=idx[:, :1], axis=0))
                    pst = gps.tile([128, 128], f32, tag="pst")
                    nc.tensor.transpose(out=pst[:], in_=xt[:], identity=ident[:])
                    nc.vector.tensor_copy(xT[:, n * 128:(n + 1) * 128], pst[:])

            def xg1_pair(p):
                # pair p = tiles {p, 7-p}; ph1 body i consumes 1f tile i and
                # 1b tile 7-i, i.e. pair min(i, 7-i).
                for tI in (p, 7 - p):
                    gather_tile(tI)
                    for u in ("1f", "1b"):
                        xg_tile(nc, u, tI, l1_rhs, wx1[u], wa1[u], aug_sb,
                                xgd[u], xps, xsb)

            xg1_pair(0)
            xg1_pair(1)

            def pre_body1(i):
                if i + 2 <= 3:
                    xg1_pair(i + 2)

            phase(nc, tc, ("1f", "1b"), wh_sb, hcar, ccar, hs, o2f, xgd,
                  identb, pre_body=pre_body1)

        # ---- xg2 (consumes hs1f/hs1b)
        def l2_rhs(k, n):
            src = hs["1f"] if k < 2 else hs["1b"]
            return src[:, n * 64:(n + 1) * 64, k % 2, :]

        with nc.named_scope("xg2"), \
             tc.tile_pool(name="xg2w", bufs=1) as xwp, \
             tc.tile_pool(name="xp2", bufs=4, space="PSUM") as xps, \
             tc.tile_pool(name="xs2", bufs=2) as xsb:
            wx2, wa2 = {}, {}
            for u in ("2f", "2b"):
                wx2[u] = xwp.tile([128, KCNT[u] * MCNT[u] * 128], bf16,
                                  tag=f"wx{u}", name=f"wx2{u}")
                nc.sync.dma_start(wx2[u][:], wxd[u][:])
                wa2[u] = xwp.tile([2, MCNT[u] * 128], bf16, tag=f"wa{u}",
                                  name=f"wa2{u}")
                nc.sync.dma_start(wa2[u][:], wad[u][:])

            def xg2_pair(p):
                # pair p = {2f tile p, 2b tile 7-p}; ph2 body i consumes
                # 2f tile i and 2b tile 7-i, i.e. exactly pair i.
                xg_tile(nc, "2f", p, l2_rhs, wx2["2f"], wa2["2f"], aug_sb,
                        xgd["2f"], xps, xsb)
                xg_tile(nc, "2b", 7 - p, l2_rhs, wx2["2b"], wa2["2b"], aug_sb,
                        xgd["2b"], xps, xsb)

            xg2_pair(0)
            xg2_pair(1)

            def pre_body2(i):
                if i + 2 < NBODY:
                    xg2_pair(i + 2)

            phase(nc, tc, ("2f", "2b"), wh_sb, hcar, ccar, hs, o2f, xgd,
                  identb, pre_body=pre_body2)

        # ---- classifier
        with nc.named_scope("cls"), \
             tc.tile_pool(name="cl", bufs=3) as cp, \
             tc.tile_pool(name="cps", bufs=3, space="PSUM") as cps:
            for n in range(TB // 512):
                psm = cps.tile([TAGS, 512], f32, tag="ps")
                for k in range(4):
                    if k < 2:
                        rhs = o2f[:, 1 + n * 64:1 + (n + 1) * 64, k, :]
                    else:
                        rhs = hs["2b"][:, n * 64:(n + 1) * 64, k % 2, :]
                    nc.tensor.matmul(
                        out=psm[:],
                        lhsT=cls_sb[:, k * TAGS:(k + 1) * TAGS],
                        rhs=rhs,
                        start=(k == 0), stop=(k == 3))
                lg = cp.tile([TAGS, 512], f32, tag="lg")
                nc.vector.tensor_scalar_add(lg[:], psm[:], clsb_sb[:, :1])
                nc.sync.dma_start(logits_d[:, n * 512:(n + 1) * 512], lg[:])

    nc.compile()
    return nc


def xg_tile(nc, u, tI, rhs_of_k, wx_sb, wa_sb, aug_sb, xg_dram, xps, xsb):
    """One 512-token tile of the xg precompute (t in [64*tI, 64*tI+64))."""
    m_cnt, k_cnt = MCNT[u], KCNT[u]
    n = tI
    nsl = slice(n * 512, (n + 1) * 512)
    stg = xsb.tile([128, m_cnt, 512], bf16, tag=f"stg{u}", name=f"stg{u}")
    for m in range(m_cnt):
        psm = xps.tile([128, 512], f32, tag="ps")
        first = True
        if m < 8:  # gate chunks get the x contribution
            for k in range(k_cnt):
                nc.tensor.matmul(
                    out=psm[:],
                    lhsT=wx_sb[:, (k * m_cnt + m) * 128:(k * m_cnt + m + 1) * 128],
                    rhs=rhs_of_k(k, n),
                    start=first, stop=False)
                first = False
        nc.tensor.matmul(
            out=psm[:],
            lhsT=wa_sb[:, m * 128:(m + 1) * 128],
            rhs=aug_sb[:, nsl],
            start=first, stop=True)
        if (n + m) % 2 == 0:
            nc.vector.tensor_copy(stg[:, m, :], psm[:])
        else:
            nc.scalar.activation(stg[:, m, :], psm[:],
                                 mybir.ActivationFunctionType.Copy)
    nc.sync.dma_start(
        xg_dram[tI][:, :, :, :],
        stg[:, :, :].rearrange("p m (t b) -> p m t b", b=Bc))


def phase(nc, tc, units, wh_sb, hcar, ccar, hs, o2f, xgd, identb,
          pre_body=None):
    """Static-unrolled recurrence for two direction units, step-interleaved."""
    MC = {u: 10 if u == "2f" else 8 for u in units}
    with nc.named_scope(f"ph{units[0]}"), \
         tc.tile_pool(name=f"rc{units[0]}", bufs=2) as rp, \
         tc.tile_pool(name=f"rps{units[0]}", bufs=2, space="PSUM") as rps, \
         tc.tile_pool(name=f"rtmp{units[0]}", bufs=4) as tp, \
         tc.tile_pool(name=f"rsc{units[0]}", bufs=2) as scp:

        def body_t0(u, i):
            return (NBODY - 1 - i) * SPB if REV[u] else i * SPB

        def dma_xb(u, i):
            xbt = rp.tile([128, MC[u], SPB, Bc], bf16, tag=f"xb{u}",
                          name=f"xb{u}")
            tI = (NBODY - 1 - i) if REV[u] else i
            nc.sync.dma_start(xbt[:, :, :, :], xgd[u][tI][:, :, :, :])
            return xbt

        def prefill(psm, u, xbt, slot):
            nc.tensor.matmul(out=psm[:, :, :], lhsT=identb[:],
                             rhs=xbt[:, 0:8, slot, :],
                             start=True, stop=False, skip_group_check=True)

        xb = {u: dma_xb(u, 0) for u in units}
        xb_next = {}
        psum = {}
        for u in units:
            slot0 = (SPB - 1) if REV[u] else 0
            ps = rps.tile([128, 8, Bc], f32, tag=f"ps{u}", name=f"ps{u}")
            prefill(ps, u, xb[u], slot0)
            psum[u] = ps

        hstage_prev = {}
        deferred = []
        for i in range(NBODY):
            if pre_body is not None:
                pre_body(i)
            hstage = {}
            for u in units:
                hstage[u] = rp.tile([128, SPB, 2, Bc], bf16, tag=f"hst{u}",
                                    name=f"hst{u}")
            for us in range(SPB):
                for u in units:
                    rev = REV[u]
                    slot = (SPB - 1 - us) if rev else us
                    if us == 0:
                        if i == 0:
                            hprev = hcar[u]
                        else:
                            pl = 0 if rev else SPB - 1
                            hprev = hstage_prev[u][:, pl, :, :]
                    else:
                        pslot = slot + 1 if rev else slot - 1
                        hprev = hstage[u][:, pslot, :, :]
                    psm = psum[u]
                    # 12 MMs for i,f,g chunks; sig(ifg) can fire while the
                    # 4 o-chunk MMs still run
                    for m in range(6):
                        for k in range(2):
                            nc.tensor.matmul(
                                out=psm[:, m, :],
                                lhsT=wh_sb[u][:, (k * 8 + m) * 128:(k * 8 + m + 1) * 128],
                                rhs=hprev[:, k, :],
                                start=False, stop=(k == 1),
                                skip_group_check=True)
                    for m in range(6, 8):
                        for k in range(2):
                            nc.tensor.matmul(
                                out=psm[:, m, :],
                                lhsT=wh_sb[u][:, (k * 8 + m) * 128:(k * 8 + m + 1) * 128],
                                rhs=hprev[:, k, :],
                                start=False, stop=(k == 1),
                                skip_group_check=True)
                    # prefetch next body's xb a bit into this body
                    if us == 4 and u == units[0] and i + 1 < NBODY:
                        for uu in units:
                            xb_next[uu] = dma_xb(uu, i + 1)
                    # prefill the NEXT step's PSUM (h-independent)
                    nxt_i = False
                    if us + 1 < SPB:
                        nslot = (slot - 1) if rev else (slot + 1)
                        nxb = xb[u]
                    elif i + 1 < NBODY:
                        nslot = (SPB - 1) if rev else 0
                        nxb = xb_next[u]
                        nxt_i = True
                    else:
                        nxb = None
                    if nxb is not None:
                        ps = rps.tile([128, 8, Bc], f32, tag=f"ps{u}",
                                      name=f"ps{u}")
                        prefill(ps, u, nxb, nslot)
                        psum[u] = ps
                    # pointwise tail (all-sigmoid)
                    sg = tp.tile([128, 8, Bc], f32, tag=f"sg{u}", name=f"sg{u}")
                    nc.scalar.activation(sg[:, 0:6, :], psm[:, 0:6, :], SIG)
                    nc.scalar.activation(sg[:, 6:8, :], psm[:, 6:8, :], SIG)
                    u1 = tp.tile([128, 2, Bc], f32, tag=f"u1{u}", name=f"u1{u}")
                    nc.vector.scalar_tensor_tensor(
                        out=u1[:, :, :], in0=sg[:, 4:6, :], scalar=-0.5,
                        in1=sg[:, 0:2, :], op0=ADD, op1=MUL)
                    csf = tp.tile([128, 2, Bc], f32, tag=f"csf{u}", name=f"csf{u}")
                    nc.vector.tensor_tensor(out=csf[:, :, :], in0=sg[:, 2:4, :],
                                            in1=ccar[u][:, :, :], op=MUL)
                    nc.vector.tensor_tensor(out=ccar[u][:, :, :],
                                            in0=csf[:, :, :], in1=u1[:, :, :],
                                            op=ADD)
                    sc = tp.tile([128, 2, Bc], f32, tag=f"sc{u}", name=f"sc{u}")
                    nc.scalar.activation(sc[:, :, :], ccar[u][:, :, :],
                                         SIG, scale=4.0)
                    nc.vector.scalar_tensor_tensor(
                        out=hstage[u][:, slot, 0, :], in0=sc[:, 0, :],
                        scalar=-0.5, in1=sg[:, 6, :], op0=ADD, op1=MUL)
                    nc.vector.scalar_tensor_tensor(
                        out=hstage[u][:, slot, 1, :], in0=sc[:, 1, :],
                        scalar=-0.5, in1=sg[:, 7, :], op0=ADD, op1=MUL)
                if deferred and us % 4 == 3:
                    deferred.pop(0)()
            # ---- body end: flush history; 2f: scan-hold
            for u in units:
                rev = REV[u]
                t0 = body_t0(u, i)
                if u != "2f":
                    nc.gpsimd.tensor_copy(hs[u][:, t0:t0 + SPB, :, :],
                                          hstage[u][:, :, :, :])
                if u == "2f":
                    # held[t] = (1-m)[t]*held[t-1] + m[t]*h[t] along t.
                    # Planes are snapshotted (GpSimd) to decouple from the xb
                    # ring; the scans themselves are DEFERRED and spread over
                    # the next body's step slots to keep the DVE queue clear.
                    mpl = scp.tile([128, 2, SPB, Bc], bf16, tag="mpl", name="mpl")
                    nc.gpsimd.tensor_copy(mpl[:, :, :, :], xb[u][:, 8:10, :, :])
                    hst2f = hstage[u]
                    for k in range(2):
                        tmp = scp.tile([128, SPB, Bc], bf16, tag=f"tmp{k}",
                                      name=f"tmp{k}")
                        nc.gpsimd.tensor_tensor(out=tmp[:, :, :],
                                                in0=hst2f[:, :, k, :],
                                                in1=mpl[:, 0, :, :], op=MUL)
                        for b in range(Bc):
                            def mk_scan(k=k, b=b, t0=t0, tmp=tmp, mpl=mpl):
                                nc.vector.tensor_tensor_scan(
                                    out=o2f[:, 1 + t0:1 + t0 + SPB, k, b],
                                    data0=mpl[:, 1, :, b],
                                    data1=tmp[:, :, b],
                                    initial=o2f[:, t0, k, b:b + 1],
                                    op0=MUL, op1=ADD)
                            deferred.append(mk_scan)
            if i + 1 < NBODY:
                xb = dict(xb_next)
            hstage_prev = hstage
        for fn in deferred:
            fn()


def _make_in_maps(inputs):
    words = np.asarray(inputs["words"]).astype(np.int32)
    lengths = np.asarray(inputs["lengths"]).astype(np.int32)
    emb = np.asarray(inputs["emb"], dtype=np.float32)
    mask = (lengths[:, None] > np.arange(T)[None, :]).astype(np.float32)
    wprep = {u: _prep_unit_weights(inputs[f"l{u}_Wih"], inputs[f"l{u}_Whh"],
                                   inputs[f"l{u}_bih"], inputs[f"l{u}_bhh"],
                                   MCNT[u], 2.0 if u[0] == "2" else 1.0)
             for u in UNITS}
    clsW = np.asarray(inputs["cls_W"], np.float64) * 2.0
    CT = clsW.T
    clsx = np.concatenate([CT[k * 128:(k + 1) * 128, :] for k in range(4)],
                          axis=1).astype(ml_dtypes.bfloat16)
    clsb = np.asarray(inputs["cls_b"], dtype=np.float32).reshape(TAGS, 1)
    in_maps = []
    for c in range(NCORES):
        bsl = slice(c * Bc, (c + 1) * Bc)
        w_c = words[bsl]
        m_c = mask[bsl]
        words_tm = np.ascontiguousarray(w_c.T).reshape(TB, 1)
        aug = np.stack([(1.0 - m_c.T).reshape(TB), np.ones(TB, np.float32)]
                       ).astype(ml_dtypes.bfloat16)
        im = {"emb": emb, "words": words_tm, "aug": aug,
              "clsx": clsx, "clsb": clsb}
        for u in UNITS:
            wx, wa, wh = wprep[u]
            im[f"w{u}x"] = wx
            im[f"w{u}a"] = wa
            im[f"w{u}h"] = wh
        in_maps.append(im)
    return in_maps


def kernel(**inputs):
    if "nc" not in _CACHE:
        _CACHE["nc"] = _build_program()
    nc = _CACHE["nc"]
    in_maps = _make_in_maps(inputs)
    _CACHE["in_maps"] = in_maps
    res = run_bass_kernel_spmd(nc, in_maps, list(range(NCORES)))
    out = np.empty((B, T, TAGS), np.float32)
    for c in range(NCORES):
        lg = res.results[c]["logits"]          # [50, TB], col = t*Bc + b
        out[c * Bc:(c + 1) * Bc] = lg.reshape(TAGS, T, Bc).transpose(2, 1, 0)
    return out


def bench(inputs):
    """Run once with NTFF tracing; returns HW exec_time_ns (and stashes trace)."""
    kernel(**inputs)  # ensure program built/cached
    nc = _CACHE["nc"]
    in_maps = _CACHE["in_maps"]
    import tempfile
    tmpdir = tempfile.mkdtemp(prefix="bilstm_trace_")
    res = run_bass_kernel_spmd(nc, in_maps, list(range(NCORES)), trace=True,
                               tmpdir=tmpdir)
    _CACHE["trace_dir"] = tmpdir
    _CACHE["last_bench"] = res
    print("trace dir:", tmpdir)
    if res.per_core_scope_times:
        for scope, times in res.per_core_scope_times.items():
            print(f"scope {scope}: {times}")
    return res.exec_time_ns


if __name__ == "__main__":
    import reference
    inputs = {k: np.asarray(v) for k, v in reference.setup_inputs().items()}
    got = kernel(**inputs)
    print(got.shape, got.dtype)


# revision 15
# speedup vs baseline: 1.3368x; 1.0076x over previous
"""BiLSTM tagger on 8 TRN2 NeuronCores.

Strategy (hardcoded for B=64,T=512,V=30000,E=128,H=256,TAGS=50):
  - Data-parallel: batch sharded 8 ways (8 sequences/core); weights replicated.
  - Per core: embedding gather (indirect DMA) -> PE transpose -> x^T in SBUF;
    input projections xg = W_ih_aug @ [x; 1-m; 1] precomputed for all t as big
    matmuls into DRAM scratch (middle-out tile order so ph1 can start after
    2 tiles); recurrences fully STATICALLY UNROLLED (no dynamic loops), xb
    DMAs prefetched one 64-step body ahead; classifier.
  - ALL-SIGMOID formulation: tanh(x) = 2*sigmoid(2x)-1 folded into weight
    scaling. Carried state is h/2, c/2; consumers' weights pre-scaled by 2
    (Whh, l2 Wih, cls_W), g-gate rows by an extra 2. Every activation is
    Sigmoid (tanh(c) = sig(4*(c/2)) via the activation scale arg) -> zero
    act-table switches.
  - Per unit-step: DVE prefills PSUM with xg (h-independent, off critical
    path); 12 Whh matmuls for i,f,g chunks then sig(ifg) fires while the 4
    o-chunk matmuls still run; DVE u1=(sg-.5)*si, csf=sf*c', c'=csf+u1;
    Act sig(4c'); DVE h'=(sc-.5)*so -> staged bf16.
  - Masking: +/-60*(1-m) on i/f gate pre-activations freezes c exactly at
    masked steps; backward h is exactly 0 there. The l2f held output is
    reconstructed per-body with tensor_tensor_scan along t:
    held[t] = (1-m[t])*held[t-1] + m[t]*h[t].
"""
import sys

sys.path.insert(0, "/opt/trn_rl_repo")
import contextlib

import numpy as np
import ml_dtypes

import concourse.bass as bass
import concourse.bacc as bacc
import concourse.mybir as mybir
import concourse.tile as tile
from concourse.bass_utils import run_bass_kernel_spmd
from concourse.masks import make_identity

B, T, V, E, H, TAGS = 64, 512, 30000, 128, 256, 50
NCORES = 8
Bc = B // NCORES          # 8 sequences per core
TB = T * Bc               # 4096 tokens per core
SPB = 64                  # steps per body
NBODY = T // SPB          # 8

f32 = mybir.dt.float32
bf16 = mybir.dt.bfloat16
i32 = mybir.dt.int32

UNITS = ("1f", "1b", "2f", "2b")
KCNT = {"1f": 1, "1b": 1, "2f": 4, "2b": 4}       # 128-row K chunks of x features
MCNT = {"1f": 8, "1b": 8, "2f": 10, "2b": 8}      # 128-row output chunks
REV = {"1f": False, "1b": True, "2f": False, "2b": True}

_CACHE = {}

SIG = mybir.ActivationFunctionType.Sigmoid
ADD = mybir.AluOpType.add
MUL = mybir.AluOpType.mult

# gate-row order [i(0:256), f(256:512), g(512:768), o(768:1024)] = torch order
# with g and o swapped
PERM = np.concatenate([np.arange(0, 512), np.arange(512, 768),
                       np.arange(768, 1024)])  # identity on i,f; then g; then o


def _prep_unit_weights(Wih, Whh, bih, bhh, m_cnt, in_scale):
    """Host-side weight marshalling (all-sigmoid form).

    Torch row order is [i f g o]; we keep it (i=chunks0:2, f=2:4, g=4:6,
    o=6:8). in_scale compensates h/2-scaled inputs (2.0 for l2). Whh x2
    (recurrent h is h/2); g rows an extra x2 (tanh = 2*sig(2x)-1)."""
    din = Wih.shape[1]
    Wp = np.asarray(Wih, np.float64) * in_scale   # [1024, din]
    Up = np.asarray(Whh, np.float64) * 2.0        # [1024, 256]
    bp = (np.asarray(bih, np.float64) + np.asarray(bhh, np.float64)).copy()
    Wp = Wp.copy()
    Wp[512:768] *= 2.0
    Up = Up.copy()
    Up[512:768] *= 2.0
    bp[512:768] *= 2.0
    M = m_cnt * 128
    k_cnt = din // 128
    # x-part lhsT: [din, M] -> k-chunk-major cols [128, k_cnt*M]
    WT = np.zeros((din, M), np.float64)
    WT[:, :1024] = Wp.T
    wx = np.concatenate([WT[k * 128:(k + 1) * 128, :] for k in range(k_cnt)],
                        axis=1).astype(ml_dtypes.bfloat16)  # [128, k_cnt*M]
    # aug lhsT rows: feature0 = (1-m), feature1 = 1
    wa = np.zeros((2, M), np.float64)
    wa[0, 0:256] = -60.0   # i rows: -60*(1-m)
    wa[0, 256:512] = 60.0  # f rows: +60*(1-m)
    wa[1, :1024] = bp
    if m_cnt == 10:        # l2f extra planes: chunk8 = m, chunk9 = 1-m
        wa[0, 1024:1152] = -1.0
        wa[1, 1024:1152] = 1.0
        wa[0, 1152:1280] = 1.0
        wa[1, 1152:1280] = 0.0
    wa = wa.astype(ml_dtypes.bfloat16)
    # Whh lhsT: [256, 1024] -> [128, 2*1024], (k*8+m) chunk indexing
    UT = Up.T
    wh = np.concatenate([UT[0:128, :], UT[128:256, :]], axis=1).astype(ml_dtypes.bfloat16)
    return wx, wa, wh


def _build_program():
    nc = bacc.Bacc("TRN2", target_bir_lowering=False, debug=False, num_devices=NCORES)
    emb_d = nc.dram_tensor("emb", [V, E], f32, kind="ExternalInput")
    words_d = nc.dram_tensor("words", [TB, 1], i32, kind="ExternalInput")
    aug_d = nc.dram_tensor("aug", [2, TB], bf16, kind="ExternalInput")
    wxd, wad, whd, xgd = {}, {}, {}, {}
    for u in UNITS:
        wxd[u] = nc.dram_tensor(f"w{u}x", [128, KCNT[u] * MCNT[u] * 128], bf16, kind="ExternalInput")
        wad[u] = nc.dram_tensor(f"w{u}a", [2, MCNT[u] * 128], bf16, kind="ExternalInput")
        whd[u] = nc.dram_tensor(f"w{u}h", [128, 2048], bf16, kind="ExternalInput")
        xgd[u] = [nc.dram_tensor(f"xg{u}t{t}", [128, MCNT[u], SPB, Bc], bf16)
                  for t in range(NBODY)]
    clsx_d = nc.dram_tensor("clsx", [128, 4 * TAGS], bf16, kind="ExternalInput")
    clsb_d = nc.dram_tensor("clsb", [TAGS, 1], f32, kind="ExternalInput")
    logits_d = nc.dram_tensor("logits", [TAGS, TB], f32, kind="ExternalOutput")

    ctx = contextlib.ExitStack()
    with tile.TileContext(nc) as tc, ctx:
        pp = ctx.enter_context(tc.tile_pool(name="persist", bufs=1))
        aug_sb = pp.tile([2, TB], bf16, tag="aug")
        wh_sb = {u: pp.tile([128, 2048], bf16, tag=f"wh{u}", name=f"wh{u}") for u in UNITS}
        cls_sb = pp.tile([128, 4 * TAGS], bf16, tag="clsx")
        clsb_sb = pp.tile([TAGS, 1], f32, tag="clsb")
        hs = {u: pp.tile([128, T, 2, Bc], bf16, tag=f"hs{u}", name=f"hs{u}")
              for u in ("1f", "1b", "2b")}
        o2f = pp.tile([128, T + 1, 2, Bc], bf16, tag="o2f")  # col0 = zeros
        identb = pp.tile([128, 128], bf16, tag="identb")
        hcar = {u: pp.tile([128, 2, Bc], bf16, tag=f"hc{u}", name=f"hc{u}") for u in UNITS}
        ccar = {u: pp.tile([128, 2, Bc], f32, tag=f"cc{u}", name=f"cc{u}") for u in UNITS}

        for u in UNITS:
            nc.sync.dma_start(wh_sb[u][:], whd[u][:])
        nc.sync.dma_start(cls_sb[:], clsx_d[:])
        nc.sync.dma_start(clsb_sb[:], clsb_d[:])
        nc.sync.dma_start(aug_sb[:], aug_d[:])
        make_identity(nc, identb[:])
        for u in UNITS:
            nc.vector.memset(hcar[u][:, :, :], 0.0)
            nc.vector.memset(ccar[u][:, :, :], 0.0)
        nc.vector.memset(o2f[:, 0, :, :], 0.0)

        # ---- prologue: gather + xg1 interleaved, middle-out tile order
        with nc.named_scope("gather"), \
             tc.tile_pool(name="prolog", bufs=1) as lp, \
             tc.tile_pool(name="gat", bufs=4) as gp, \
             tc.tile_pool(name="gps", bufs=2, space="PSUM") as gps, \
             tc.tile_pool(name="xp1", bufs=2, space="PSUM") as xps, \
             tc.tile_pool(name="xs1", bufs=2) as xsb:
            xT = lp.tile([128, TB], bf16, tag="xT")
            ident = lp.tile([128, 128], f32, tag="ident")
            make_identity(nc, ident[:])
            wx1, wa1 = {}, {}
            for u in ("1f", "1b"):
                wx1[u] = lp.tile([128, KCNT[u] * MCNT[u] * 128], bf16,
                                 tag=f"wx{u}", name=f"wx{u}")
                nc.sync.dma_start(wx1[u][:], wxd[u][:])
                wa1[u] = lp.tile([2, MCNT[u] * 128], bf16, tag=f"wa{u}",
                                 name=f"wa{u}")
                nc.sync.dma_start(wa1[u][:], wad[u][:])

            def l1_rhs(k, n):
                return xT[:, n * 512:(n + 1) * 512]

            def gather_tile(tI):
                for j in range(4):
                    n = tI * 4 + j
                    idx = gp.tile([128, 1], i32, tag="idx")
                    nc.sync.dma_start(idx[:], words_d[n * 128:(n + 1) * 128, :])
                    xt = gp.tile([128, 128], f32, tag="xt")
                    nc.gpsimd.indirect_dma_start(
                        out=xt[:], out_offset=None, in_=emb_d[:, :],
                        in_offset=bass.IndirectOffsetOnAxis(ap=idx[:, :1], axis=0))
                    pst = gps.tile([128, 128], f32, tag="pst")
                    nc.tensor.transpose(out=pst[:], in_=xt[:], identity=ident[:])
                    nc.vector.tensor_copy(xT[:, n * 128:(n + 1) * 128], pst[:])

            def xg1_pair(p):
                # pair p = tiles {p, 7-p}; ph1 body i consumes 1f tile i and
                # 1b tile 7-i, i.e. pair min(i, 7-i).
                for tI in (p, 7 - p):
                    gather_tile(tI)
                    for u in ("1f", "1b"):
                        xg_tile(nc, u, tI, l1_rhs, wx1[u], wa1[u], aug_sb,
                                xgd[u], xps, xsb)

            xg1_pair(0)
            xg1_pair(1)

            def pre_body1(i):
                if i + 2 <= 3:
                    xg1_pair(i + 2)

            phase(nc, tc, ("1f", "1b"), wh_sb, hcar, ccar, hs, o2f, xgd,
                  identb, pre_body=pre_body1)

        # ---- xg2 (consumes hs1f/hs1b)
        def l2_rhs(k, n):
            src = hs["1f"] if k < 2 else hs["1b"]
            return src[:, n * 64:(n + 1) * 64, k % 2, :]

        with nc.named_scope("xg2"), \
             tc.tile_pool(name="xg2w", bufs=1) as xwp, \
             tc.tile_pool(name="xp2", bufs=4, space="PSUM") as xps, \
             tc.tile_pool(name="xs2", bufs=2) as xsb:
            wx2, wa2 = {}, {}
            for u in ("2f", "2b"):
                wx2[u] = xwp.tile([128, KCNT[u] * MCNT[u] * 128], bf16,
                                  tag=f"wx{u}", name=f"wx2{u}")
                nc.sync.dma_start(wx2[u][:], wxd[u][:])
                wa2[u] = xwp.tile([2, MCNT[u] * 128], bf16, tag=f"wa{u}",
                                  name=f"wa2{u}")
                nc.sync.dma_start(wa2[u][:], wad[u][:])

            def xg2_pair(p):
                # pair p = {2f tile p, 2b tile 7-p}; ph2 body i consumes
                # 2f tile i and 2b tile 7-i, i.e. exactly pair i.
                xg_tile(nc, "2f", p, l2_rhs, wx2["2f"], wa2["2f"], aug_sb,
                        xgd["2f"], xps, xsb)
                xg_tile(nc, "2b", 7 - p, l2_rhs, wx2["2b"], wa2["2b"], aug_sb,
                        xgd["2b"], xps, xsb)

            xg2_pair(0)
            xg2_pair(1)

            def pre_body2(i):
                if i + 2 < NBODY:
                    xg2_pair(i + 2)

            phase(nc, tc, ("2f", "2b"), wh_sb, hcar, ccar, hs, o2f, xgd,
                  identb, pre_body=pre_body2)

        # ---- classifier
        with nc.named_scope("cls"), \
             tc.tile_pool(name="cl", bufs=3) as cp, \
             tc.tile_pool(name="cps", bufs=3, space="PSUM") as cps:
            for n in range(TB // 512):
                psm = cps.tile([TAGS, 512], f32, tag="ps")
                for k in range(4):
                    if k < 2:
                        rhs = o2f[:, 1 + n * 64:1 + (n + 1) * 64, k, :]
                    else:
                        rhs = hs["2b"][:, n * 64:(n + 1) * 64, k % 2, :]
                    nc.tensor.matmul(
                        out=psm[:],
                        lhsT=cls_sb[:, k * TAGS:(k + 1) * TAGS],
                        rhs=rhs,
                        start=(k == 0), stop=(k == 3))
                lg = cp.tile([TAGS, 512], f32, tag="lg")
                nc.vector.tensor_scalar_add(lg[:], psm[:], clsb_sb[:, :1])
                nc.sync.dma_start(logits_d[:, n * 512:(n + 1) * 512], lg[:])

    nc.compile()
    return nc


def xg_tile(nc, u, tI, rhs_of_k, wx_sb, wa_sb, aug_sb, xg_dram, xps, xsb):
    """One 512-token tile of the xg precompute (t in [64*tI, 64*tI+64))."""
    m_cnt, k_cnt = MCNT[u], KCNT[u]
    n = tI
    nsl = slice(n * 512, (n + 1) * 512)
    stg = xsb.tile([128, m_cnt, 512], bf16, tag=f"stg{u}", name=f"stg{u}")
    for m in range(m_cnt):
        psm = xps.tile([128, 512], f32, tag="ps")
        first = True
        if m < 8:  # gate chunks get the x contribution
            for k in range(k_cnt):
                nc.tensor.matmul(
                    out=psm[:],
                    lhsT=wx_sb[:, (k * m_cnt + m) * 128:(k * m_cnt + m + 1) * 128],
                    rhs=rhs_of_k(k, n),
                    start=first, stop=False)
                first = False
        nc.tensor.matmul(
            out=psm[:],
            lhsT=wa_sb[:, m * 128:(m + 1) * 128],
            rhs=aug_sb[:, nsl],
            start=first, stop=True)
        if (n + m) % 2 == 0:
            nc.vector.tensor_copy(stg[:, m, :], psm[:])
        else:
            nc.scalar.activation(stg[:, m, :], psm[:],
                                 mybir.ActivationFunctionType.Copy)
    nc.sync.dma_start(
        xg_dram[tI][:, :, :, :],
        stg[:, :, :].rearrange("p m (t b) -> p m t b", b=Bc))


def phase(nc, tc, units, wh_sb, hcar, ccar, hs, o2f, xgd, identb,
          pre_body=None):
    """Static-unrolled recurrence for two direction units.

    Unit B is emitted ONE FULL STEP behind unit A: in the in-order PE stream
    A-sweep(t+1) precedes B-sweep(t), so each unit's pointwise tail overlaps
    the other unit's sweep instead of locksteping (tail-block + sweep-block).
    """
    MC = {u: 10 if u == "2f" else 8 for u in units}
    with nc.named_scope(f"ph{units[0]}"), \
         tc.tile_pool(name=f"rc{units[0]}", bufs=2) as rp, \
         tc.tile_pool(name=f"rps{units[0]}", bufs=2, space="PSUM") as rps, \
         tc.tile_pool(name=f"rtmp{units[0]}", bufs=4) as tp, \
         tc.tile_pool(name=f"rsc{units[0]}", bufs=2) as scp:

        def body_t0(u, i):
            return (NBODY - 1 - i) * SPB if REV[u] else i * SPB

        def dma_xb(u, i):
            xbt = rp.tile([128, MC[u], SPB, Bc], bf16, tag=f"xb{u}",
                          name=f"xb{u}")
            tI = (NBODY - 1 - i) if REV[u] else i
            nc.sync.dma_start(xbt[:, :, :, :], xgd[u][tI][:, :, :, :])
            return xbt

        def prefill(psm, u, xbt, slot):
            nc.tensor.matmul(out=psm[:, :, :], lhsT=identb[:],
                             rhs=xbt[:, 0:8, slot, :],
                             start=True, stop=False, skip_group_check=True)

        xb = {u: dma_xb(u, 0) for u in units}
        xb_next = {}
        psum = {}
        for u in units:
            slot0 = (SPB - 1) if REV[u] else 0
            ps = rps.tile([128, 8, Bc], f32, tag=f"ps{u}", name=f"ps{u}")
            prefill(ps, u, xb[u], slot0)
            psum[u] = ps

        hstage = {}
        hstage_prev = {}
        deferred = []

        def emit_step(u, ugs):
            i, us = divmod(ugs, SPB)
            rev = REV[u]
            slot = (SPB - 1 - us) if rev else us
            if us == 0:
                if u == units[0] and pre_body is not None:
                    pre_body(i)
                hstage[u] = rp.tile([128, SPB, 2, Bc], bf16, tag=f"hst{u}",
                                    name=f"hst{u}")
            if us == 0:
                if i == 0:
                    hprev = hcar[u]
                else:
                    pl = 0 if rev else SPB - 1
                    hprev = hstage_prev[u][:, pl, :, :]
            else:
                pslot = slot + 1 if rev else slot - 1
                hprev = hstage[u][:, pslot, :, :]
            psm = psum[u]
            # 12 MMs for i,f,g chunks; sig(ifg) fires while o-chunk MMs run
            for m in range(8):
                for k in range(2):
                    nc.tensor.matmul(
                        out=psm[:, m, :],
                        lhsT=wh_sb[u][:, (k * 8 + m) * 128:(k * 8 + m + 1) * 128],
                        rhs=hprev[:, k, :],
                        start=False, stop=(k == 1),
                        skip_group_check=True)
            # per-unit prefetch of the next body's xb
            if us == 4 and i + 1 < NBODY:
                xb_next[u] = dma_xb(u, i + 1)
            # prefill the NEXT step's PSUM (h-independent)
            nxb = None
            if us + 1 < SPB:
                nslot = (slot - 1) if rev else (slot + 1)
                nxb = xb[u]
            elif i + 1 < NBODY:
                nslot = (SPB - 1) if rev else 0
                nxb = xb_next[u]
            if nxb is not None:
                ps = rps.tile([128, 8, Bc], f32, tag=f"ps{u}", name=f"ps{u}")
                prefill(ps, u, nxb, nslot)
                psum[u] = ps
            # pointwise tail (all-sigmoid)
            sg = tp.tile([128, 8, Bc], f32, tag=f"sg{u}", name=f"sg{u}")
            nc.scalar.activation(sg[:, 0:6, :], psm[:, 0:6, :], SIG)
            nc.scalar.activation(sg[:, 6:8, :], psm[:, 6:8, :], SIG)
            u1 = tp.tile([128, 2, Bc], f32, tag=f"u1{u}", name=f"u1{u}")
            nc.vector.scalar_tensor_tensor(
                out=u1[:, :, :], in0=sg[:, 4:6, :], scalar=-0.5,
                in1=sg[:, 0:2, :], op0=ADD, op1=MUL)
            csf = tp.tile([128, 2, Bc], f32, tag=f"csf{u}", name=f"csf{u}")
            nc.vector.tensor_tensor(out=csf[:, :, :], in0=sg[:, 2:4, :],
                                    in1=ccar[u][:, :, :], op=MUL)
            nc.vector.tensor_tensor(out=ccar[u][:, :, :],
                                    in0=csf[:, :, :], in1=u1[:, :, :], op=ADD)
            sc = tp.tile([128, 2, Bc], f32, tag=f"sc{u}", name=f"sc{u}")
            nc.scalar.activation(sc[:, :, :], ccar[u][:, :, :], SIG, scale=4.0)
            nc.vector.scalar_tensor_tensor(
                out=hstage[u][:, slot, 0, :], in0=sc[:, 0, :],
                scalar=-0.5, in1=sg[:, 6, :], op0=ADD, op1=MUL)
            nc.vector.scalar_tensor_tensor(
                out=hstage[u][:, slot, 1, :], in0=sc[:, 1, :],
                scalar=-0.5, in1=sg[:, 7, :], op0=ADD, op1=MUL)
            if u == units[0] and deferred and us % 4 == 3:
                deferred.pop(0)()
            if us == SPB - 1:
                end_body(u, i)

        def end_body(u, i):
            t0 = body_t0(u, i)
            if u != "2f":
                nc.gpsimd.tensor_copy(hs[u][:, t0:t0 + SPB, :, :],
                                      hstage[u][:, :, :, :])
            else:
                # held[t] = (1-m)[t]*held[t-1] + m[t]*h[t] along t.
                # Planes snapshotted (GpSimd) to decouple from the xb ring;
                # scans DEFERRED and spread over later step slots.
                mpl = scp.tile([128, 2, SPB, Bc], bf16, tag="mpl", name="mpl")
                nc.gpsimd.tensor_copy(mpl[:, :, :, :], xb[u][:, 8:10, :, :])
                hst2f = hstage[u]
                for k in range(2):
                    tmp = scp.tile([128, SPB, Bc], bf16, tag=f"tmp{k}",
                                   name=f"tmp{k}")
                    nc.gpsimd.tensor_tensor(out=tmp[:, :, :],
                                            in0=hst2f[:, :, k, :],
                                            in1=mpl[:, 0, :, :], op=MUL)
                    for b in range(Bc):
                        def mk_scan(k=k, b=b, t0=t0, tmp=tmp, mpl=mpl):
                            nc.vector.tensor_tensor_scan(
                                out=o2f[:, 1 + t0:1 + t0 + SPB, k, b],
                                data0=mpl[:, 1, :, b],
                                data1=tmp[:, :, b],
                                initial=o2f[:, t0, k, b:b + 1],
                                op0=MUL, op1=ADD)
                        deferred.append(mk_scan)
            if i + 1 < NBODY:
                xb[u] = xb_next[u]
            hstage_prev[u] = hstage[u]

        # unit B trails unit A by exactly one step
        for gs in range(T + 1):
            if gs < T:
                emit_step(units[0], gs)
            if gs >= 1:
                emit_step(units[1], gs - 1)
        for fn in deferred:
            fn()


def _make_in_maps(inputs):
    words = np.asarray(inputs["words"]).astype(np.int32)
    lengths = np.asarray(inputs["lengths"]).astype(np.int32)
    emb = np.asarray(inputs["emb"], dtype=np.float32)
    mask = (lengths[:, None] > np.arange(T)[None, :]).astype(np.float32)
    wprep = {u: _prep_unit_weights(inputs[f"l{u}_Wih"], inputs[f"l{u}_Whh"],
                                   inputs[f"l{u}_bih"], inputs[f"l{u}_bhh"],
                                   MCNT[u], 2.0 if u[0] == "2" else 1.0)
             for u in UNITS}
    clsW = np.asarray(inputs["cls_W"], np.float64) * 2.0
    CT = clsW.T
    clsx = np.concatenate([CT[k * 128:(k + 1) * 128, :] for k in range(4)],
                          axis=1).astype(ml_dtypes.bfloat16)
    clsb = np.asarray(inputs["cls_b"], dtype=np.float32).reshape(TAGS, 1)
    in_maps = []
    for c in range(NCORES):
        bsl = slice(c * Bc, (c + 1) * Bc)
        w_c = words[bsl]
        m_c = mask[bsl]
        words_tm = np.ascontiguousarray(w_c.T).reshape(TB, 1)
        aug = np.stack([(1.0 - m_c.T).reshape(TB), np.ones(TB, np.float32)]
                       ).astype(ml_dtypes.bfloat16)
        im = {"emb": emb, "words": words_tm, "aug": aug,
              "clsx": clsx, "clsb": clsb}
        for u in UNITS:
            wx, wa, wh = wprep[u]
            im[f"w{u}x"] = wx
            im[f"w{u}a"] = wa
            im[f"w{u}h"] = wh
        in_maps.append(im)
    return in_maps


def kernel(**inputs):
    if "nc" not in _CACHE:
        _CACHE["nc"] = _build_program()
    nc = _CACHE["nc"]
    in_maps = _make_in_maps(inputs)
    _CACHE["in_maps"] = in_maps
    res = run_bass_kernel_spmd(nc, in_maps, list(range(NCORES)))
    out = np.empty((B, T, TAGS), np.float32)
    for c in range(NCORES):
        lg = res.results[c]["logits"]          # [50, TB], col = t*Bc + b
        out[c * Bc:(c + 1) * Bc] = lg.reshape(TAGS, T, Bc).transpose(2, 1, 0)
    return out


def bench(inputs):
    """Run once with NTFF tracing; returns HW exec_time_ns (and stashes trace)."""
    kernel(**inputs)  # ensure program built/cached
    nc = _CACHE["nc"]
    in_maps = _CACHE["in_maps"]
    import tempfile
    tmpdir = tempfile.mkdtemp(prefix="bilstm_trace_")
    res = run_bass_kernel_spmd(nc, in_maps, list(range(NCORES)), trace=True,
                               tmpdir=tmpdir)
    _CACHE["trace_dir"] = tmpdir
    _CACHE["last_bench"] = res
    print("trace dir:", tmpdir)
    if res.per_core_scope_times:
        for scope, times in res.per_core_scope_times.items():
            print(f"scope {scope}: {times}")
    return res.exec_time_ns


if __name__ == "__main__":
    import reference
    inputs = {k: np.asarray(v) for k, v in reference.setup_inputs().items()}
    got = kernel(**inputs)
    print(got.shape, got.dtype)
